# revision 33
# baseline (speedup 1.0000x reference)
"""ChamferkNNDist kernel for Trainium2 (8 NeuronCores, pure data parallel).

Reference math (per batch element b, K=4096 points, 3 dims):
  chamfer_b = mean_i min_j ||adv_i - ori_j||^2
  dd_ij     = ||adv_i - adv_j||^2
  value_i   = mean of the 5 smallest dd_ij excluding self
  knn_b     = mean_i value_i * [value_i > mean(value) + 1.05*std(value, ddof=1)]
  loss      = 5 * mean_b chamfer_b + 3 * mean_b knn_b

Device strategy (one batch element per core):
  The PE emits NEGATED squared distances directly: the 13-row bf16
  contraction computes -d_ij = 2 a_i.b_j - |b_j|^2 - |a_i|^2 with every
  fp32 factor compensated-split into bf16 hi+lo (dropped lo*lo cross terms
  leave ~1e-4 abs error; the row-constant |a_i|^2 rides along as two extra
  lhsT rows against all-ones rhs rows, so the cancellation happens in fp32
  PSUM). Row pairing (lhsT x rhs), with A = 2a:
    k0-2: Ah.bh   k3-5: Al.bh   k6-8: Ah.bl   k9,10: 1*(-bb hi,lo)
    k11,12: (-aa hi,lo)*1
  Streaming cost is 1 column/cycle regardless of the 13 rows.

  Per 128-query chunk, PSUM holds -d in two f32 [128,2048] halves.
  Real-hardware constraints discovered by probing the walrus verifier and
  the device: GPSIMD cannot touch PSUM or run max ALU ops, DMA cannot read
  PSUM, matmul output must be f32, and tensor_tensor_reduce faults at
  runtime -- so DVE carries all compare work:
  - kNN: DVE max8 directly on each PSUM half (top-8 of -d; rank 1 = self
    at ~0), a [128,16] merge max8, value_i = -mean(ranks 2..6).
  - chamfer: ACT drains halves to a bf16 [128,4096] row (relative
    precision is preserved because -d is small near the min); DVE runs a
    bf16 2x-mode pairwise-max tree + small reduce, emitted two chunks
    late so it never gates the max8s. D = -max(-d).
  Chamfer matmuls trail the kNN matmuls by one chunk in the PE stream,
  and a PE warm-up stream ramps the clock before the transposes.
  Batch stats (mean/std/threshold/masked mean) on device via ones-matmul
  column sums; host only averages the 8 per-core (chamfer_b, knn_b) pairs.
"""

import os
import sys
from contextlib import ExitStack

import numpy as np

try:
    import concourse  # noqa: F401
except ImportError:  # staged repo location inside the container
    for _p in ("/opt/trn_rl_repo", os.path.expanduser("~/.axon_site/_ro/trn_rl_repo")):
        if os.path.isdir(_p):
            sys.path.insert(0, _p)
            break

import concourse.bacc as bacc
import concourse.tile as tile
from concourse import mybir

F32 = mybir.dt.float32
BF16 = mybir.dt.bfloat16
ALU = mybir.AluOpType
AX = mybir.AxisListType

NPTS = 4096
N_CORES = 8
K_NN = 5
ALPHA = 1.05
W_CHAMFER = 5.0
W_KNN = 3.0
NEG_INF = -3.0e38
NROW = 13


def build_body(tc, ctx: ExitStack, adv, ori, out, npts):
    nc = tc.nc
    nch = npts // 128

    singles = ctx.enter_context(tc.tile_pool(name="singles", bufs=1))
    prep = ctx.enter_context(tc.tile_pool(name="prep", bufs=1))
    feat = ctx.enter_context(tc.tile_pool(name="feat", bufs=1))
    acc = ctx.enter_context(tc.tile_pool(name="acc", bufs=1))
    d16p = ctx.enter_context(tc.tile_pool(name="d16p", bufs=3))
    champ = ctx.enter_context(tc.tile_pool(name="champ", bufs=3))
    scrp = ctx.enter_context(tc.tile_pool(name="scrp", bufs=3))
    small = ctx.enter_context(tc.tile_pool(name="small", bufs=3))

    # ---------------- identity + PE warm-up ----------------
    # The PE clock ramps with sustained use; stream throwaway matmuls while
    # the DMA + staging prep runs so the transposes and first chunks start
    # at full speed.
    ident_i = singles.tile([128, 128], mybir.dt.int32, tag="identI")
    nc.gpsimd.iota(ident_i[:], pattern=[[1, 128]], base=0, channel_multiplier=-1)
    ident = singles.tile([128, 128], BF16, tag="ident")
    nc.vector.tensor_scalar(ident[:], ident_i[:], 0.0, None, op0=ALU.is_equal)
    wrm = singles.tile([128, 512], BF16, tag="wrm")
    nc.gpsimd.memset(wrm[:], 0.5)
    with tc.tile_pool(name="wpsum", bufs=1, space="PSUM") as wpsum:
        wps = wpsum.tile([128, 512], F32, tag="wps")
        for _ in range(12):
            nc.tensor.matmul(wps[:], wrm[:, 0:128], wrm[:], start=True, stop=True)

    # ---------------- load points (contiguous; point order is a
    # permutation, and every reduction here is permutation-invariant) ------
    P_a = prep.tile([128, nch, 3], F32, tag="P_a")
    nc.sync.dma_start(out=P_a[:], in_=adv.rearrange("(p c) d -> p c d", c=nch))
    P_o = prep.tile([128, nch, 3], F32, tag="P_o")
    nc.sync.dma_start(out=P_o[:], in_=ori.rearrange("(p c) d -> p c d", c=nch))

    # ---------------- negated squared norms ----------------
    def norms(P, tag):
        sq = prep.tile([128, nch, 3], F32, tag=f"sq{tag}")
        nc.vector.tensor_mul(sq[:], P[:], P[:])
        nn = prep.tile([128, nch, 1], F32, tag=f"nn{tag}")
        nc.vector.tensor_reduce(nn[:], sq[:], axis=AX.X, op=ALU.add)
        ng = prep.tile([128, nch, 1], F32, tag=f"ng{tag}")
        nc.vector.tensor_scalar_mul(ng[:], nn[:], -1.0)
        return nn, ng

    aa, naa = norms(P_a, "a")   # aa = |a|^2,  naa = -aa
    _bb, nbb = norms(P_o, "o")

    # naa bf16 hi/lo split, shared by S_L rows 11,12 and S_RA rows 9,10
    sh3 = [128, nch, 3]
    sh1 = [128, nch, 1]
    nah = prep.tile(sh1, BF16, tag="nah")
    nal = prep.tile(sh1, BF16, tag="nal")
    nc.scalar.copy(nah[:], naa[:])
    r0 = prep.tile(sh1, F32, tag="r0")
    nc.vector.tensor_sub(r0[:], naa[:], nah[:])
    nc.scalar.copy(nal[:], r0[:])

    # ---------------- bf16 hi/lo staging, point-major [128, nch, 13] ------
    S_L = prep.tile([128, nch, NROW], BF16, tag="S_L")
    # lhsT rows: Ah(0:3), Al(3:6), Ah dup(6:9), 1(9:11), nah(11), nal(12)
    B2 = prep.tile(sh3, F32, tag="B2")
    nc.vector.tensor_scalar_mul(B2[:], P_a[:], 2.0)
    nc.scalar.copy(S_L[:, :, 0:3], B2[:])                     # Ah = bf16(2a)
    rl = prep.tile(sh3, F32, tag="rl")
    nc.vector.tensor_sub(rl[:], B2[:], S_L[:, :, 0:3])
    nc.scalar.copy(S_L[:, :, 3:6], rl[:])                     # Al
    nc.vector.tensor_copy(S_L[:, :, 6:9], S_L[:, :, 0:3])
    nc.gpsimd.memset(S_L[:, :, 9:11], 1.0)
    nc.vector.tensor_copy(S_L[:, :, 11:12], nah[:])
    nc.vector.tensor_copy(S_L[:, :, 12:13], nal[:])

    def build_rhs(P, nh_src, nl_src, ng, tag, eng, cast):
        # rhs rows: bh(0:3), bh dup(3:6), bl(6:9), nb hi(9), nb lo(10),
        # ones(11:13). Chain on one engine so the two rhs builds overlap.
        S = prep.tile([128, nch, NROW], BF16, tag=f"S_{tag}")
        cast(S[:, :, 0:3], P[:])                              # bh
        r2 = prep.tile(sh3, F32, tag=f"r2_{tag}")
        eng.tensor_sub(r2[:], P[:], S[:, :, 0:3])
        cast(S[:, :, 6:9], r2[:])                             # bl
        eng.tensor_copy(S[:, :, 3:6], S[:, :, 0:3])
        if nh_src is not None:
            eng.tensor_copy(S[:, :, 9:10], nh_src[:])
            eng.tensor_copy(S[:, :, 10:11], nl_src[:])
        else:
            cast(S[:, :, 9:10], ng[:])                        # nb hi
            r3 = prep.tile(sh1, F32, tag=f"r3_{tag}")
            eng.tensor_sub(r3[:], ng[:], S[:, :, 9:10])
            cast(S[:, :, 10:11], r3[:])                       # nb lo
        nc.gpsimd.memset(S[:, :, 11:13], 1.0)
        return S

    S_RA = build_rhs(P_a, nah, nal, None, "ra", nc.gpsimd,
                     lambda o, i: nc.gpsimd.tensor_copy(o, i))
    S_RO = build_rhs(P_o, None, None, nbb, "ro", nc.vector,
                     lambda o, i: nc.scalar.copy(o, i))

    # ---------------- transpose staging -> feature-major [13, npts] -------
    T_L = feat.tile([NROW, npts], BF16, tag="T_L")
    T_RA = feat.tile([NROW, npts], BF16, tag="T_RA")
    T_RO = feat.tile([NROW, npts], BF16, tag="T_RO")

    S5 = acc.tile([128, nch], F32, tag="S5")   # sum of -d ranks 2..6 (knn)
    MU = acc.tile([128, nch], F32, tag="MU")   # chamfer row max of -d

    act_cp = lambda o, i: nc.scalar.copy(o, i)        # noqa: E731
    dve_cp = lambda o, i: nc.vector.tensor_copy(o, i)  # noqa: E731

    # knn: two max8 straight off the f32 -d PSUM halves (no drain).
    # cham: ACT drains halves to a bf16 [128,4096] row; one DVE ttr
    # (trailing two chunks) reduces it. Pool/GPSIMD cannot max on real HW,
    # and ttr/DMA cannot touch PSUM, so DVE carries all compare work.
    with tc.tile_pool(name="tpsum", bufs=2, space="PSUM") as tpsum, \
         tc.tile_pool(name="k0dist", bufs=1, space="PSUM") as k0dist:

        def tgroup(S, T, g, drain):
            pt = tpsum.tile([NROW, 1024], BF16, tag="pt")
            for ci in range(8):
                c = g * 8 + ci
                nc.tensor.transpose(
                    pt[:, ci * 128:(ci + 1) * 128], S[:, c, :], ident[:]
                )
            drain(T[:, g * 1024:(g + 1) * 1024], pt[:])

        for g in range(4):
            tgroup(S_L, T_L, g, dve_cp)
        for g in range(4):
            tgroup(S_RA, T_RA, g, act_cp if g < 2 else dve_cp)

        # chunk 0 kNN via a single-buffered tile so it overlaps the T_RO
        # transposes (the main dist pool needs all 8 banks)
        u16_0 = small.tile([128, 16], F32, tag="u16")
        lhsT0 = T_L[:, 0:128]
        for h in range(2):
            kh = k0dist.tile([128, 2048], F32, tag="k0")
            for q in range(4):
                j0 = h * 2048 + q * 512
                nc.tensor.matmul(kh[:, q * 512:(q + 1) * 512], lhsT0,
                                 T_RA[:, j0:j0 + 512], start=True, stop=True)
            nc.vector.max(out=u16_0[:, h * 8:(h + 1) * 8], in_=kh[:])
        for g in range(4):
            tgroup(S_RO, T_RO, g, act_cp)

    def dve_knn(c, u16):
        u8 = small.tile([128, 8], F32, tag="u8")
        nc.vector.max(out=u8[:], in_=u16[:])
        nc.vector.tensor_reduce(S5[:, c:c + 1], u8[:, 1:6], axis=AX.X,
                                op=ALU.add)

    def dve_ttr(c, D16c):
        # emitted two chunks late so DVE never stalls on ACT's drains.
        # tensor_tensor_reduce faults on real trn2, so this is a bf16
        # 2x-mode pairwise-max tree (the instruction mix the baseline
        # proved on hardware) plus one small reduce.
        t1 = scrp.tile([128, 2048], BF16, tag="t1")
        nc.vector.tensor_tensor(t1[:], D16c[:, 0:2048], D16c[:, 2048:4096],
                                op=ALU.max)
        t2 = scrp.tile([128, 1024], BF16, tag="t2")
        nc.vector.tensor_tensor(t2[:], t1[:, 0:1024], t1[:, 1024:2048],
                                op=ALU.max)
        t3 = scrp.tile([128, 512], BF16, tag="t3")
        nc.vector.tensor_tensor(t3[:], t2[:, 0:512], t2[:, 512:1024],
                                op=ALU.max)
        t4 = scrp.tile([128, 256], BF16, tag="t4")
        nc.vector.tensor_tensor(t4[:], t3[:, 0:256], t3[:, 256:512],
                                op=ALU.max)
        nc.vector.tensor_reduce(MU[:, c:c + 1], t4[:], axis=AX.X, op=ALU.max)

    with tc.tile_pool(name="dist", bufs=2, space="PSUM") as dist:

        def knn_half(c, h, u16):
            lhsT = T_L[:, c * 128:(c + 1) * 128]
            kh = dist.tile([128, 2048], F32, tag="d")
            for q in range(4):
                j0 = h * 2048 + q * 512
                nc.tensor.matmul(kh[:, q * 512:(q + 1) * 512], lhsT,
                                 T_RA[:, j0:j0 + 512], start=True, stop=True)
            nc.vector.max(out=u16[:, h * 8:(h + 1) * 8], in_=kh[:])

        def cham_half(c, h, D16c):
            lhsT = T_L[:, c * 128:(c + 1) * 128]
            ch = dist.tile([128, 2048], F32, tag="d")
            for q in range(4):
                j0 = h * 2048 + q * 512
                nc.tensor.matmul(ch[:, q * 512:(q + 1) * 512], lhsT,
                                 T_RO[:, j0:j0 + 512], start=True, stop=True)
            nc.scalar.copy(D16c[:, h * 2048:(h + 1) * 2048], ch[:])

        dve_knn(0, u16_0)
        D16s = {}
        prevD = d16p.tile([128, npts], BF16, tag="D16c")
        for c in range(1, nch):
            u16 = small.tile([128, 16], F32, tag="u16")
            knn_half(c, 0, u16)
            knn_half(c, 1, u16)
            cham_half(c - 1, 0, prevD)
            cham_half(c - 1, 1, prevD)
            D16s[c - 1] = prevD
            prevD = d16p.tile([128, npts], BF16, tag="D16c")
            dve_knn(c, u16)
            if c >= 2:
                dve_ttr(c - 2, D16s.pop(c - 2))
        cham_half(nch - 1, 0, prevD)
        cham_half(nch - 1, 1, prevD)
        dve_ttr(nch - 2, D16s.pop(nch - 2))
        dve_ttr(nch - 1, prevD)

    # ---------------- finalize: per-batch scalars ----------------
    ones = singles.tile([128, 1], F32, tag="ones")
    nc.vector.memset(ones[:], 1.0)

    D = acc.tile([128, nch], F32, tag="D")     # chamfer min distances
    nc.vector.tensor_scalar_mul(D[:], MU[:], -1.0)
    VAL = acc.tile([128, nch], F32, tag="VAL")  # knn value_i
    nc.vector.tensor_scalar_mul(VAL[:], S5[:], -1.0 / K_NN)
    V2 = acc.tile([128, nch], F32, tag="V2")
    nc.vector.tensor_mul(V2[:], VAL[:], VAL[:])

    n = float(npts)
    st = small.tile([1, 12], F32, tag="st")
    outsb = small.tile([1, 2], F32, tag="outsb")
    with tc.tile_pool(name="cspsum", bufs=1, space="PSUM") as csp:
        cs = csp.tile([1, 3 * nch], F32, tag="cs")
        nc.tensor.matmul(cs[:, 0:nch], ones[:], D[:], start=True, stop=True)
        nc.tensor.matmul(cs[:, nch:2 * nch], ones[:], VAL[:], start=True, stop=True)
        nc.tensor.matmul(cs[:, 2 * nch:3 * nch], ones[:], V2[:], start=True, stop=True)
        nc.vector.tensor_reduce(
            st[:, 0:3], cs[:].rearrange("o (g x) -> o g x", g=3),
            axis=AX.X, op=ALU.add,
        )

        # chamfer_b
        nc.vector.tensor_scalar_mul(outsb[:, 0:1], st[:, 0:1], 1.0 / n)
        # value stats: mean, var (ddof=1), threshold
        nc.vector.tensor_scalar_mul(st[:, 3:4], st[:, 1:2], 1.0 / n)          # mean
        nc.vector.tensor_mul(st[:, 4:5], st[:, 1:2], st[:, 1:2])              # sumV^2
        nc.vector.tensor_scalar_mul(st[:, 5:6], st[:, 4:5], 1.0 / n)
        nc.vector.tensor_sub(st[:, 6:7], st[:, 2:3], st[:, 5:6])
        nc.vector.tensor_scalar_mul(st[:, 7:8], st[:, 6:7], 1.0 / (n - 1.0))  # var
        nc.scalar.sqrt(st[:, 8:9], st[:, 7:8])                                # std
        nc.vector.tensor_scalar_mul(st[:, 9:10], st[:, 8:9], ALPHA)
        nc.vector.tensor_add(st[:, 10:11], st[:, 3:4], st[:, 9:10])           # thr

        thrb = small.tile([128, 1], F32, tag="thrb")
        nc.gpsimd.partition_broadcast(thrb[:], st[:, 10:11])
        G = acc.tile([128, nch], F32, tag="G")
        nc.vector.tensor_scalar(G[:], VAL[:], thrb[:, 0:1], None, op0=ALU.is_gt)
        VM = acc.tile([128, nch], F32, tag="VM")
        nc.vector.tensor_mul(VM[:], VAL[:], G[:])
        cs2 = csp.tile([1, nch], F32, tag="cs2")
        nc.tensor.matmul(cs2[:, 0:nch], ones[:], VM[:], start=True, stop=True)
        nc.vector.tensor_reduce(st[:, 11:12], cs2[:, 0:nch], axis=AX.X, op=ALU.add)
        nc.vector.tensor_scalar_mul(outsb[:, 1:2], st[:, 11:12], 1.0 / n)

    nc.sync.dma_start(out=out[0:1, 0:2], in_=outsb[:])


def build_nc(npts=NPTS):
    nc = bacc.Bacc("TRN2", target_bir_lowering=False, debug=False)
    adv = nc.dram_tensor("adv", [npts, 3], F32, kind="ExternalInput")
    ori = nc.dram_tensor("ori", [npts, 3], F32, kind="ExternalInput")
    out = nc.dram_tensor("out", [1, 2], F32, kind="ExternalOutput")
    with tile.TileContext(nc) as tc, ExitStack() as ctx:
        build_body(tc, ctx, adv.ap(), ori.ap(), out.ap(), npts)
    nc.compile()
    return nc


_NC_CACHE = {}


def _get_nc(npts=NPTS):
    if npts not in _NC_CACHE:
        _NC_CACHE[npts] = build_nc(npts)
    return _NC_CACHE[npts]


def kernel(**inputs) -> np.ndarray:
    from concourse.bass_utils import run_bass_kernel_spmd

    adv = np.ascontiguousarray(np.asarray(inputs["adv_pc"], dtype=np.float32))
    ori = np.ascontiguousarray(np.asarray(inputs["ori_pc"], dtype=np.float32))
    B = adv.shape[0]
    assert B == N_CORES and adv.shape[1] == NPTS, (adv.shape, ori.shape)

    nc = _get_nc()
    in_maps = [{"adv": adv[b], "ori": ori[b]} for b in range(B)]
    res = run_bass_kernel_spmd(nc, in_maps, core_ids=list(range(N_CORES)))
    parts = np.stack([r["out"][0] for r in res.results])  # [B, 2]
    loss = W_CHAMFER * parts[:, 0].mean() + W_KNN * parts[:, 1].mean()
    return np.float32(loss)


# revision 35
# speedup vs baseline: 1.0076x; 1.0076x over previous
"""ChamferkNNDist kernel for Trainium2 (8 NeuronCores, pure data parallel).

Reference math (per batch element b, K=4096 points, 3 dims):
  chamfer_b = mean_i min_j ||adv_i - ori_j||^2
  dd_ij     = ||adv_i - adv_j||^2
  value_i   = mean of the 5 smallest dd_ij excluding self
  knn_b     = mean_i value_i * [value_i > mean(value) + 1.05*std(value, ddof=1)]
  loss      = 5 * mean_b chamfer_b + 3 * mean_b knn_b

Device strategy (one batch element per core):
  The PE emits NEGATED squared distances directly: the 13-row bf16
  contraction computes -d_ij = 2 a_i.b_j - |b_j|^2 - |a_i|^2 with every
  fp32 factor compensated-split into bf16 hi+lo (dropped lo*lo cross terms
  leave ~1e-4 abs error; the row-constant |a_i|^2 rides along as two extra
  lhsT rows against all-ones rhs rows, so the cancellation happens in fp32
  PSUM). Row pairing (lhsT x rhs), with A = 2a:
    k0-2: Ah.bh   k3-5: Al.bh   k6-8: Ah.bl   k9,10: 1*(-bb hi,lo)
    k11,12: (-aa hi,lo)*1
  Streaming cost is 1 column/cycle regardless of the 13 rows.

  Per 128-query chunk, PSUM holds -d in two f32 [128,2048] halves.
  Real-hardware constraints discovered by probing the walrus verifier and
  the device: GPSIMD cannot touch PSUM or run max ALU ops, DMA cannot read
  PSUM, matmul output must be f32, and tensor_tensor_reduce faults at
  runtime -- so DVE carries all compare work:
  - kNN: DVE max8 directly on each PSUM half (top-8 of -d; rank 1 = self
    at ~0), a [128,16] merge max8, value_i = -mean(ranks 2..6).
  - chamfer: ACT drains halves to a bf16 [128,4096] row (relative
    precision is preserved because -d is small near the min); DVE runs a
    bf16 2x-mode pairwise-max tree + small reduce, emitted two chunks
    late so it never gates the max8s. D = -max(-d).
  Chamfer matmuls trail the kNN matmuls by one chunk in the PE stream,
  and a PE warm-up stream ramps the clock before the transposes.
  Batch stats (mean/std/threshold/masked mean) on device via ones-matmul
  column sums; host only averages the 8 per-core (chamfer_b, knn_b) pairs.
"""

import os
import sys
from contextlib import ExitStack

import numpy as np

try:
    import concourse  # noqa: F401
except ImportError:  # staged repo location inside the container
    for _p in ("/opt/trn_rl_repo", os.path.expanduser("~/.axon_site/_ro/trn_rl_repo")):
        if os.path.isdir(_p):
            sys.path.insert(0, _p)
            break

import concourse.bacc as bacc
import concourse.tile as tile
from concourse import mybir

F32 = mybir.dt.float32
BF16 = mybir.dt.bfloat16
ALU = mybir.AluOpType
AX = mybir.AxisListType

NPTS = 4096
N_CORES = 8
K_NN = 5
ALPHA = 1.05
W_CHAMFER = 5.0
W_KNN = 3.0
NEG_INF = -3.0e38
NROW = 13


def build_body(tc, ctx: ExitStack, adv, ori, out, npts):
    nc = tc.nc
    nch = npts // 128

    singles = ctx.enter_context(tc.tile_pool(name="singles", bufs=1))
    prep = ctx.enter_context(tc.tile_pool(name="prep", bufs=1))
    feat = ctx.enter_context(tc.tile_pool(name="feat", bufs=1))
    acc = ctx.enter_context(tc.tile_pool(name="acc", bufs=1))
    d16p = ctx.enter_context(tc.tile_pool(name="d16p", bufs=3))
    champ = ctx.enter_context(tc.tile_pool(name="champ", bufs=3))
    scrp = ctx.enter_context(tc.tile_pool(name="scrp", bufs=3))
    small = ctx.enter_context(tc.tile_pool(name="small", bufs=3))

    # ---------------- identity + PE warm-up ----------------
    # The PE clock ramps with sustained use; stream throwaway matmuls while
    # the DMA + staging prep runs so the transposes and first chunks start
    # at full speed.
    ident_i = singles.tile([128, 128], mybir.dt.int32, tag="identI")
    nc.gpsimd.iota(ident_i[:], pattern=[[1, 128]], base=0, channel_multiplier=-1)
    ident = singles.tile([128, 128], BF16, tag="ident")
    nc.vector.tensor_scalar(ident[:], ident_i[:], 0.0, None, op0=ALU.is_equal)
    wrm = singles.tile([128, 512], BF16, tag="wrm")
    nc.gpsimd.memset(wrm[:], 0.5)
    with tc.tile_pool(name="wpsum", bufs=1, space="PSUM") as wpsum:
        wps = wpsum.tile([128, 512], F32, tag="wps")
        for _ in range(12):
            nc.tensor.matmul(wps[:], wrm[:, 0:128], wrm[:], start=True, stop=True)

    # ---------------- load points (contiguous; point order is a
    # permutation, and every reduction here is permutation-invariant) ------
    P_a = prep.tile([128, nch, 3], F32, tag="P_a")
    nc.sync.dma_start(out=P_a[:], in_=adv.rearrange("(p c) d -> p c d", c=nch))
    P_o = prep.tile([128, nch, 3], F32, tag="P_o")
    nc.sync.dma_start(out=P_o[:], in_=ori.rearrange("(p c) d -> p c d", c=nch))

    # ---------------- negated squared norms ----------------
    def norms(P, tag):
        sq = prep.tile([128, nch, 3], F32, tag=f"sq{tag}")
        nc.vector.tensor_mul(sq[:], P[:], P[:])
        nn = prep.tile([128, nch, 1], F32, tag=f"nn{tag}")
        nc.vector.tensor_reduce(nn[:], sq[:], axis=AX.X, op=ALU.add)
        ng = prep.tile([128, nch, 1], F32, tag=f"ng{tag}")
        nc.vector.tensor_scalar_mul(ng[:], nn[:], -1.0)
        return nn, ng

    aa, naa = norms(P_a, "a")   # aa = |a|^2,  naa = -aa
    _bb, nbb = norms(P_o, "o")

    # naa bf16 hi/lo split, shared by S_L rows 11,12 and S_RA rows 9,10
    sh3 = [128, nch, 3]
    sh1 = [128, nch, 1]
    nah = prep.tile(sh1, BF16, tag="nah")
    nal = prep.tile(sh1, BF16, tag="nal")
    nc.scalar.copy(nah[:], naa[:])
    r0 = prep.tile(sh1, F32, tag="r0")
    nc.vector.tensor_sub(r0[:], naa[:], nah[:])
    nc.scalar.copy(nal[:], r0[:])

    # ---------------- bf16 hi/lo staging, point-major [128, nch, 13] ------
    S_L = prep.tile([128, nch, NROW], BF16, tag="S_L")
    # lhsT rows: Ah(0:3), Al(3:6), Ah dup(6:9), 1(9:11), nah(11), nal(12)
    B2 = prep.tile(sh3, F32, tag="B2")
    nc.vector.tensor_scalar_mul(B2[:], P_a[:], 2.0)
    nc.scalar.copy(S_L[:, :, 0:3], B2[:])                     # Ah = bf16(2a)
    rl = prep.tile(sh3, F32, tag="rl")
    nc.vector.tensor_sub(rl[:], B2[:], S_L[:, :, 0:3])
    nc.scalar.copy(S_L[:, :, 3:6], rl[:])                     # Al
    nc.vector.tensor_copy(S_L[:, :, 6:9], S_L[:, :, 0:3])
    nc.gpsimd.memset(S_L[:, :, 9:11], 1.0)
    nc.vector.tensor_copy(S_L[:, :, 11:12], nah[:])
    nc.vector.tensor_copy(S_L[:, :, 12:13], nal[:])

    def build_rhs(P, nh_src, nl_src, ng, tag, eng, cast):
        # rhs rows: bh(0:3), bh dup(3:6), bl(6:9), nb hi(9), nb lo(10),
        # ones(11:13). Chain on one engine so the two rhs builds overlap.
        S = prep.tile([128, nch, NROW], BF16, tag=f"S_{tag}")
        cast(S[:, :, 0:3], P[:])                              # bh
        r2 = prep.tile(sh3, F32, tag=f"r2_{tag}")
        eng.tensor_sub(r2[:], P[:], S[:, :, 0:3])
        cast(S[:, :, 6:9], r2[:])                             # bl
        eng.tensor_copy(S[:, :, 3:6], S[:, :, 0:3])
        if nh_src is not None:
            eng.tensor_copy(S[:, :, 9:10], nh_src[:])
            eng.tensor_copy(S[:, :, 10:11], nl_src[:])
        else:
            cast(S[:, :, 9:10], ng[:])                        # nb hi
            r3 = prep.tile(sh1, F32, tag=f"r3_{tag}")
            eng.tensor_sub(r3[:], ng[:], S[:, :, 9:10])
            cast(S[:, :, 10:11], r3[:])                       # nb lo
        nc.gpsimd.memset(S[:, :, 11:13], 1.0)
        return S

    S_RA = build_rhs(P_a, nah, nal, None, "ra", nc.gpsimd,
                     lambda o, i: nc.gpsimd.tensor_copy(o, i))
    S_RO = build_rhs(P_o, None, None, nbb, "ro", nc.vector,
                     lambda o, i: nc.vector.tensor_copy(o, i))

    # ---------------- transpose staging -> feature-major [13, npts] -------
    T_L = feat.tile([NROW, npts], BF16, tag="T_L")
    T_RA = feat.tile([NROW, npts], BF16, tag="T_RA")
    T_RO = feat.tile([NROW, npts], BF16, tag="T_RO")

    S5 = acc.tile([128, nch], F32, tag="S5")   # sum of -d ranks 2..6 (knn)
    MU = acc.tile([128, nch], F32, tag="MU")   # chamfer row max of -d

    act_cp = lambda o, i: nc.scalar.copy(o, i)        # noqa: E731
    dve_cp = lambda o, i: nc.vector.tensor_copy(o, i)  # noqa: E731

    # knn: two max8 straight off the f32 -d PSUM halves (no drain).
    # cham: ACT drains halves to a bf16 [128,4096] row; one DVE ttr
    # (trailing two chunks) reduces it. Pool/GPSIMD cannot max on real HW,
    # and ttr/DMA cannot touch PSUM, so DVE carries all compare work.
    with tc.tile_pool(name="tpsum", bufs=4, space="PSUM") as tpsum, \
         tc.tile_pool(name="k0dist", bufs=1, space="PSUM") as k0dist:

        def tgroup(S, T, g, drain):
            pt = tpsum.tile([NROW, 1024], BF16, tag="pt")
            for ci in range(8):
                c = g * 8 + ci
                nc.tensor.transpose(
                    pt[:, ci * 128:(ci + 1) * 128], S[:, c, :], ident[:]
                )
            drain(T[:, g * 1024:(g + 1) * 1024], pt[:])

        for g in range(4):
            tgroup(S_L, T_L, g, dve_cp)
        for g in range(4):
            tgroup(S_RA, T_RA, g, act_cp if g < 2 else dve_cp)

        # chunk 0 kNN via a single-buffered tile so it overlaps the T_RO
        # transposes (the main dist pool needs all 8 banks)
        u16_0 = small.tile([128, 16], F32, tag="u16")
        lhsT0 = T_L[:, 0:128]
        for h in range(2):
            kh = k0dist.tile([128, 2048], F32, tag="k0")
            for q in range(4):
                j0 = h * 2048 + q * 512
                nc.tensor.matmul(kh[:, q * 512:(q + 1) * 512], lhsT0,
                                 T_RA[:, j0:j0 + 512], start=True, stop=True)
            nc.vector.max(out=u16_0[:, h * 8:(h + 1) * 8], in_=kh[:])
        for g in range(4):
            tgroup(S_RO, T_RO, g, act_cp)

    def dve_knn(c, u16):
        u8 = small.tile([128, 8], F32, tag="u8")
        nc.vector.max(out=u8[:], in_=u16[:])
        nc.vector.tensor_reduce(S5[:, c:c + 1], u8[:, 1:6], axis=AX.X,
                                op=ALU.add)

    def dve_ttr(c, D16c):
        # emitted two chunks late so DVE never stalls on ACT's drains.
        # tensor_tensor_reduce faults on real trn2, so this is a bf16
        # 2x-mode pairwise-max tree (the instruction mix the baseline
        # proved on hardware) plus one small reduce.
        t1 = scrp.tile([128, 2048], BF16, tag="t1")
        nc.vector.tensor_tensor(t1[:], D16c[:, 0:2048], D16c[:, 2048:4096],
                                op=ALU.max)
        t2 = scrp.tile([128, 1024], BF16, tag="t2")
        nc.vector.tensor_tensor(t2[:], t1[:, 0:1024], t1[:, 1024:2048],
                                op=ALU.max)
        t3 = scrp.tile([128, 512], BF16, tag="t3")
        nc.vector.tensor_tensor(t3[:], t2[:, 0:512], t2[:, 512:1024],
                                op=ALU.max)
        t4 = scrp.tile([128, 256], BF16, tag="t4")
        nc.vector.tensor_tensor(t4[:], t3[:, 0:256], t3[:, 256:512],
                                op=ALU.max)
        nc.vector.tensor_reduce(MU[:, c:c + 1], t4[:], axis=AX.X, op=ALU.max)

    with tc.tile_pool(name="dist", bufs=2, space="PSUM") as dist:

        def knn_half(c, h, u16):
            lhsT = T_L[:, c * 128:(c + 1) * 128]
            kh = dist.tile([128, 2048], F32, tag="d")
            for q in range(4):
                j0 = h * 2048 + q * 512
                nc.tensor.matmul(kh[:, q * 512:(q + 1) * 512], lhsT,
                                 T_RA[:, j0:j0 + 512], start=True, stop=True)
            nc.vector.max(out=u16[:, h * 8:(h + 1) * 8], in_=kh[:])

        def cham_half(c, h, D16c):
            lhsT = T_L[:, c * 128:(c + 1) * 128]
            ch = dist.tile([128, 2048], F32, tag="d")
            for q in range(4):
                j0 = h * 2048 + q * 512
                nc.tensor.matmul(ch[:, q * 512:(q + 1) * 512], lhsT,
                                 T_RO[:, j0:j0 + 512], start=True, stop=True)
            nc.scalar.copy(D16c[:, h * 2048:(h + 1) * 2048], ch[:])

        dve_knn(0, u16_0)
        D16s = {}
        prevD = d16p.tile([128, npts], BF16, tag="D16c")
        for c in range(1, nch):
            u16 = small.tile([128, 16], F32, tag="u16")
            knn_half(c, 0, u16)
            knn_half(c, 1, u16)
            cham_half(c - 1, 0, prevD)
            cham_half(c - 1, 1, prevD)
            D16s[c - 1] = prevD
            prevD = d16p.tile([128, npts], BF16, tag="D16c")
            dve_knn(c, u16)
            if c >= 2:
                dve_ttr(c - 2, D16s.pop(c - 2))
        cham_half(nch - 1, 0, prevD)
        cham_half(nch - 1, 1, prevD)
        dve_ttr(nch - 2, D16s.pop(nch - 2))
        dve_ttr(nch - 1, prevD)

    # ---------------- finalize: per-batch scalars ----------------
    ones = singles.tile([128, 1], F32, tag="ones")
    nc.vector.memset(ones[:], 1.0)

    D = acc.tile([128, nch], F32, tag="D")     # chamfer min distances
    nc.vector.tensor_scalar_mul(D[:], MU[:], -1.0)
    VAL = acc.tile([128, nch], F32, tag="VAL")  # knn value_i
    nc.vector.tensor_scalar_mul(VAL[:], S5[:], -1.0 / K_NN)
    V2 = acc.tile([128, nch], F32, tag="V2")
    nc.vector.tensor_mul(V2[:], VAL[:], VAL[:])

    n = float(npts)
    st = small.tile([1, 12], F32, tag="st")
    outsb = small.tile([1, 2], F32, tag="outsb")
    with tc.tile_pool(name="cspsum", bufs=1, space="PSUM") as csp:
        cs = csp.tile([1, 3 * nch], F32, tag="cs")
        nc.tensor.matmul(cs[:, 0:nch], ones[:], D[:], start=True, stop=True)
        nc.tensor.matmul(cs[:, nch:2 * nch], ones[:], VAL[:], start=True, stop=True)
        nc.tensor.matmul(cs[:, 2 * nch:3 * nch], ones[:], V2[:], start=True, stop=True)
        nc.vector.tensor_reduce(
            st[:, 0:3], cs[:].rearrange("o (g x) -> o g x", g=3),
            axis=AX.X, op=ALU.add,
        )

        # chamfer_b
        nc.vector.tensor_scalar_mul(outsb[:, 0:1], st[:, 0:1], 1.0 / n)
        # value stats: mean, var (ddof=1), threshold
        nc.vector.tensor_scalar_mul(st[:, 3:4], st[:, 1:2], 1.0 / n)          # mean
        nc.vector.tensor_mul(st[:, 4:5], st[:, 1:2], st[:, 1:2])              # sumV^2
        nc.vector.tensor_scalar_mul(st[:, 5:6], st[:, 4:5], 1.0 / n)
        nc.vector.tensor_sub(st[:, 6:7], st[:, 2:3], st[:, 5:6])
        nc.vector.tensor_scalar_mul(st[:, 7:8], st[:, 6:7], 1.0 / (n - 1.0))  # var
        nc.scalar.sqrt(st[:, 8:9], st[:, 7:8])                                # std
        nc.vector.tensor_scalar_mul(st[:, 9:10], st[:, 8:9], ALPHA)
        nc.vector.tensor_add(st[:, 10:11], st[:, 3:4], st[:, 9:10])           # thr

        thrb = small.tile([128, 1], F32, tag="thrb")
        nc.gpsimd.partition_broadcast(thrb[:], st[:, 10:11])
        G = acc.tile([128, nch], F32, tag="G")
        nc.vector.tensor_scalar(G[:], VAL[:], thrb[:, 0:1], None, op0=ALU.is_gt)
        VM = acc.tile([128, nch], F32, tag="VM")
        nc.vector.tensor_mul(VM[:], VAL[:], G[:])
        cs2 = csp.tile([1, nch], F32, tag="cs2")
        nc.tensor.matmul(cs2[:, 0:nch], ones[:], VM[:], start=True, stop=True)
        nc.vector.tensor_reduce(st[:, 11:12], cs2[:, 0:nch], axis=AX.X, op=ALU.add)
        nc.vector.tensor_scalar_mul(outsb[:, 1:2], st[:, 11:12], 1.0 / n)

    nc.sync.dma_start(out=out[0:1, 0:2], in_=outsb[:])


def build_nc(npts=NPTS):
    nc = bacc.Bacc("TRN2", target_bir_lowering=False, debug=False)
    adv = nc.dram_tensor("adv", [npts, 3], F32, kind="ExternalInput")
    ori = nc.dram_tensor("ori", [npts, 3], F32, kind="ExternalInput")
    out = nc.dram_tensor("out", [1, 2], F32, kind="ExternalOutput")
    with tile.TileContext(nc) as tc, ExitStack() as ctx:
        build_body(tc, ctx, adv.ap(), ori.ap(), out.ap(), npts)
    nc.compile()
    return nc


_NC_CACHE = {}


def _get_nc(npts=NPTS):
    if npts not in _NC_CACHE:
        _NC_CACHE[npts] = build_nc(npts)
    return _NC_CACHE[npts]


def kernel(**inputs) -> np.ndarray:
    from concourse.bass_utils import run_bass_kernel_spmd

    adv = np.ascontiguousarray(np.asarray(inputs["adv_pc"], dtype=np.float32))
    ori = np.ascontiguousarray(np.asarray(inputs["ori_pc"], dtype=np.float32))
    B = adv.shape[0]
    assert B == N_CORES and adv.shape[1] == NPTS, (adv.shape, ori.shape)

    nc = _get_nc()
    in_maps = [{"adv": adv[b], "ori": ori[b]} for b in range(B)]
    res = run_bass_kernel_spmd(nc, in_maps, core_ids=list(range(N_CORES)))
    parts = np.stack([r["out"][0] for r in res.results])  # [B, 2]
    loss = W_CHAMFER * parts[:, 0].mean() + W_KNN * parts[:, 1].mean()
    return np.float32(loss)


# revision 36
# speedup vs baseline: 1.0169x; 1.0092x over previous
"""ChamferkNNDist kernel for Trainium2 (8 NeuronCores, pure data parallel).

Reference math (per batch element b, K=4096 points, 3 dims):
  chamfer_b = mean_i min_j ||adv_i - ori_j||^2
  dd_ij     = ||adv_i - adv_j||^2
  value_i   = mean of the 5 smallest dd_ij excluding self
  knn_b     = mean_i value_i * [value_i > mean(value) + 1.05*std(value, ddof=1)]
  loss      = 5 * mean_b chamfer_b + 3 * mean_b knn_b

Device strategy (one batch element per core):
  The PE emits NEGATED squared distances directly: the 13-row bf16
  contraction computes -d_ij = 2 a_i.b_j - |b_j|^2 - |a_i|^2 with every
  fp32 factor compensated-split into bf16 hi+lo (dropped lo*lo cross terms
  leave ~1e-4 abs error; the row-constant |a_i|^2 rides along as two extra
  lhsT rows against all-ones rhs rows, so the cancellation happens in fp32
  PSUM). Row pairing (lhsT x rhs), with A = 2a:
    k0-2: Ah.bh   k3-5: Al.bh   k6-8: Ah.bl   k9,10: 1*(-bb hi,lo)
    k11,12: (-aa hi,lo)*1
  Streaming cost is 1 column/cycle regardless of the 13 rows.

  Per 128-query chunk, PSUM holds -d in two f32 [128,2048] halves.
  Real-hardware constraints discovered by probing the walrus verifier and
  the device: GPSIMD cannot touch PSUM or run max ALU ops, DMA cannot read
  PSUM, matmul output must be f32, and tensor_tensor_reduce faults at
  runtime -- so DVE carries all compare work:
  - kNN: DVE max8 directly on each PSUM half (top-8 of -d; rank 1 = self
    at ~0), a [128,16] merge max8, value_i = -mean(ranks 2..6).
  - chamfer: ACT drains halves to a bf16 [128,4096] row (relative
    precision is preserved because -d is small near the min); DVE runs a
    bf16 2x-mode pairwise-max tree + small reduce, emitted two chunks
    late so it never gates the max8s. D = -max(-d).
  Chamfer matmuls trail the kNN matmuls by one chunk in the PE stream,
  and a PE warm-up stream ramps the clock before the transposes.
  Batch stats (mean/std/threshold/masked mean) on device via ones-matmul
  column sums; host only averages the 8 per-core (chamfer_b, knn_b) pairs.
"""

import os
import sys
from contextlib import ExitStack

import numpy as np

try:
    import concourse  # noqa: F401
except ImportError:  # staged repo location inside the container
    for _p in ("/opt/trn_rl_repo", os.path.expanduser("~/.axon_site/_ro/trn_rl_repo")):
        if os.path.isdir(_p):
            sys.path.insert(0, _p)
            break

import concourse.bacc as bacc
import concourse.tile as tile
from concourse import mybir

F32 = mybir.dt.float32
BF16 = mybir.dt.bfloat16
ALU = mybir.AluOpType
AX = mybir.AxisListType

NPTS = 4096
N_CORES = 8
K_NN = 5
ALPHA = 1.05
W_CHAMFER = 5.0
W_KNN = 3.0
NEG_INF = -3.0e38
NROW = 13


def build_body(tc, ctx: ExitStack, adv, ori, out, npts):
    nc = tc.nc
    nch = npts // 128

    singles = ctx.enter_context(tc.tile_pool(name="singles", bufs=1))
    prep = ctx.enter_context(tc.tile_pool(name="prep", bufs=1))
    feat = ctx.enter_context(tc.tile_pool(name="feat", bufs=1))
    acc = ctx.enter_context(tc.tile_pool(name="acc", bufs=1))
    d16p = ctx.enter_context(tc.tile_pool(name="d16p", bufs=3))
    champ = ctx.enter_context(tc.tile_pool(name="champ", bufs=3))
    scrp = ctx.enter_context(tc.tile_pool(name="scrp", bufs=3))
    small = ctx.enter_context(tc.tile_pool(name="small", bufs=3))
    d16kp = ctx.enter_context(tc.tile_pool(name="d16kp", bufs=2))

    # ---------------- identity + PE warm-up ----------------
    # The PE clock ramps with sustained use; stream throwaway matmuls while
    # the DMA + staging prep runs so the transposes and first chunks start
    # at full speed.
    ident_i = singles.tile([128, 128], mybir.dt.int32, tag="identI")
    nc.gpsimd.iota(ident_i[:], pattern=[[1, 128]], base=0, channel_multiplier=-1)
    ident = singles.tile([128, 128], BF16, tag="ident")
    nc.vector.tensor_scalar(ident[:], ident_i[:], 0.0, None, op0=ALU.is_equal)
    wrm = singles.tile([128, 512], BF16, tag="wrm")
    nc.gpsimd.memset(wrm[:], 0.5)
    with tc.tile_pool(name="wpsum", bufs=1, space="PSUM") as wpsum:
        wps = wpsum.tile([128, 512], F32, tag="wps")
        for _ in range(12):
            nc.tensor.matmul(wps[:], wrm[:, 0:128], wrm[:], start=True, stop=True)

    # ---------------- load points (contiguous; point order is a
    # permutation, and every reduction here is permutation-invariant) ------
    P_a = prep.tile([128, nch, 3], F32, tag="P_a")
    nc.sync.dma_start(out=P_a[:], in_=adv.rearrange("(p c) d -> p c d", c=nch))
    P_o = prep.tile([128, nch, 3], F32, tag="P_o")
    nc.sync.dma_start(out=P_o[:], in_=ori.rearrange("(p c) d -> p c d", c=nch))

    # ---------------- negated squared norms ----------------
    def norms(P, tag):
        sq = prep.tile([128, nch, 3], F32, tag=f"sq{tag}")
        nc.vector.tensor_mul(sq[:], P[:], P[:])
        nn = prep.tile([128, nch, 1], F32, tag=f"nn{tag}")
        nc.vector.tensor_reduce(nn[:], sq[:], axis=AX.X, op=ALU.add)
        ng = prep.tile([128, nch, 1], F32, tag=f"ng{tag}")
        nc.vector.tensor_scalar_mul(ng[:], nn[:], -1.0)
        return nn, ng

    aa, naa = norms(P_a, "a")   # aa = |a|^2,  naa = -aa
    _bb, nbb = norms(P_o, "o")

    # naa bf16 hi/lo split, shared by S_L rows 11,12 and S_RA rows 9,10
    sh3 = [128, nch, 3]
    sh1 = [128, nch, 1]
    nah = prep.tile(sh1, BF16, tag="nah")
    nal = prep.tile(sh1, BF16, tag="nal")
    nc.scalar.copy(nah[:], naa[:])
    r0 = prep.tile(sh1, F32, tag="r0")
    nc.vector.tensor_sub(r0[:], naa[:], nah[:])
    nc.scalar.copy(nal[:], r0[:])

    # ---------------- bf16 hi/lo staging, point-major [128, nch, 13] ------
    S_L = prep.tile([128, nch, NROW], BF16, tag="S_L")
    # lhsT rows: Ah(0:3), Al(3:6), Ah dup(6:9), 1(9:11), nah(11), nal(12)
    B2 = prep.tile(sh3, F32, tag="B2")
    nc.vector.tensor_scalar_mul(B2[:], P_a[:], 2.0)
    nc.scalar.copy(S_L[:, :, 0:3], B2[:])                     # Ah = bf16(2a)
    rl = prep.tile(sh3, F32, tag="rl")
    nc.vector.tensor_sub(rl[:], B2[:], S_L[:, :, 0:3])
    nc.scalar.copy(S_L[:, :, 3:6], rl[:])                     # Al
    nc.vector.tensor_copy(S_L[:, :, 6:9], S_L[:, :, 0:3])
    nc.gpsimd.memset(S_L[:, :, 9:11], 1.0)
    nc.vector.tensor_copy(S_L[:, :, 11:12], nah[:])
    nc.vector.tensor_copy(S_L[:, :, 12:13], nal[:])

    def build_rhs(P, nh_src, nl_src, ng, tag, eng, cast):
        # rhs rows: bh(0:3), bh dup(3:6), bl(6:9), nb hi(9), nb lo(10),
        # ones(11:13). Chain on one engine so the two rhs builds overlap.
        S = prep.tile([128, nch, NROW], BF16, tag=f"S_{tag}")
        cast(S[:, :, 0:3], P[:])                              # bh
        r2 = prep.tile(sh3, F32, tag=f"r2_{tag}")
        eng.tensor_sub(r2[:], P[:], S[:, :, 0:3])
        cast(S[:, :, 6:9], r2[:])                             # bl
        eng.tensor_copy(S[:, :, 3:6], S[:, :, 0:3])
        if nh_src is not None:
            eng.tensor_copy(S[:, :, 9:10], nh_src[:])
            eng.tensor_copy(S[:, :, 10:11], nl_src[:])
        else:
            cast(S[:, :, 9:10], ng[:])                        # nb hi
            r3 = prep.tile(sh1, F32, tag=f"r3_{tag}")
            eng.tensor_sub(r3[:], ng[:], S[:, :, 9:10])
            cast(S[:, :, 10:11], r3[:])                       # nb lo
        nc.gpsimd.memset(S[:, :, 11:13], 1.0)
        return S

    S_RA = build_rhs(P_a, nah, nal, None, "ra", nc.gpsimd,
                     lambda o, i: nc.gpsimd.tensor_copy(o, i))
    S_RO = build_rhs(P_o, None, None, nbb, "ro", nc.vector,
                     lambda o, i: nc.vector.tensor_copy(o, i))

    # ---------------- transpose staging -> feature-major [13, npts] -------
    T_L = feat.tile([NROW, npts], BF16, tag="T_L")
    T_RA = feat.tile([NROW, npts], BF16, tag="T_RA")
    T_RO = feat.tile([NROW, npts], BF16, tag="T_RO")

    S5 = acc.tile([128, nch], F32, tag="S5")   # sum of -d ranks 2..6 (knn)
    MU = acc.tile([128, nch], F32, tag="MU")   # chamfer row max of -d

    act_cp = lambda o, i: nc.scalar.copy(o, i)        # noqa: E731
    dve_cp = lambda o, i: nc.vector.tensor_copy(o, i)  # noqa: E731

    # knn: two max8 straight off the f32 -d PSUM halves (no drain).
    # cham: ACT drains halves to a bf16 [128,4096] row; one DVE ttr
    # (trailing two chunks) reduces it. Pool/GPSIMD cannot max on real HW,
    # and ttr/DMA cannot touch PSUM, so DVE carries all compare work.
    with tc.tile_pool(name="tpsum", bufs=4, space="PSUM") as tpsum, \
         tc.tile_pool(name="k0dist", bufs=1, space="PSUM") as k0dist:

        def tgroup(S, T, g, drain):
            pt = tpsum.tile([NROW, 1024], BF16, tag="pt")
            for ci in range(8):
                c = g * 8 + ci
                nc.tensor.transpose(
                    pt[:, ci * 128:(ci + 1) * 128], S[:, c, :], ident[:]
                )
            drain(T[:, g * 1024:(g + 1) * 1024], pt[:])

        for g in range(4):
            tgroup(S_L, T_L, g, dve_cp)
        for g in range(4):
            tgroup(S_RA, T_RA, g, act_cp if g < 2 else dve_cp)

        # chunk 0 kNN via a single-buffered tile so it overlaps the T_RO
        # transposes (the main dist pool needs all 8 banks)
        u16_0 = small.tile([128, 16], F32, tag="u16")
        lhsT0 = T_L[:, 0:128]
        for h in range(2):
            kh = k0dist.tile([128, 2048], F32, tag="k0")
            for q in range(4):
                j0 = h * 2048 + q * 512
                nc.tensor.matmul(kh[:, q * 512:(q + 1) * 512], lhsT0,
                                 T_RA[:, j0:j0 + 512], start=True, stop=True)
            nc.vector.max(out=u16_0[:, h * 8:(h + 1) * 8], in_=kh[:])
        for g in range(4):
            tgroup(S_RO, T_RO, g, act_cp)

    def dve_knn(c, u16):
        u8 = small.tile([128, 8], F32, tag="u8")
        nc.vector.max(out=u8[:], in_=u16[:])
        nc.vector.tensor_reduce(S5[:, c:c + 1], u8[:, 1:6], axis=AX.X,
                                op=ALU.add)

    def dve_ttr(c, D16c):
        # emitted two chunks late so DVE never stalls on ACT's drains.
        # tensor_tensor_reduce faults on real trn2, so this is a bf16
        # 2x-mode pairwise-max tree (the instruction mix the baseline
        # proved on hardware) plus one small reduce.
        t1 = scrp.tile([128, 2048], BF16, tag="t1")
        nc.vector.tensor_tensor(t1[:], D16c[:, 0:2048], D16c[:, 2048:4096],
                                op=ALU.max)
        t2 = scrp.tile([128, 1024], BF16, tag="t2")
        nc.vector.tensor_tensor(t2[:], t1[:, 0:1024], t1[:, 1024:2048],
                                op=ALU.max)
        t3 = scrp.tile([128, 512], BF16, tag="t3")
        nc.vector.tensor_tensor(t3[:], t2[:, 0:512], t2[:, 512:1024],
                                op=ALU.max)
        t4 = scrp.tile([128, 256], BF16, tag="t4")
        nc.vector.tensor_tensor(t4[:], t3[:, 0:256], t3[:, 256:512],
                                op=ALU.max)
        nc.vector.tensor_reduce(MU[:, c:c + 1], t4[:], axis=AX.X, op=ALU.max)

    with tc.tile_pool(name="dist", bufs=2, space="PSUM") as dist:

        def knn_half(c, h, u16):
            # H0: max8 straight off PSUM. H1: ACT (which has slack) drains
            # to SBUF f32 first -- the SBUF max8 is cheaper and the PSUM
            # slot frees earlier.
            lhsT = T_L[:, c * 128:(c + 1) * 128]
            kh = dist.tile([128, 2048], F32, tag="d")
            for q in range(4):
                j0 = h * 2048 + q * 512
                nc.tensor.matmul(kh[:, q * 512:(q + 1) * 512], lhsT,
                                 T_RA[:, j0:j0 + 512], start=True, stop=True)
            if h == 0:
                nc.vector.max(out=u16[:, 0:8], in_=kh[:])
            else:
                dk = d16kp.tile([128, 2048], F32, tag="D16k")
                nc.scalar.copy(dk[:], kh[:])
                nc.vector.max(out=u16[:, 8:16], in_=dk[:])

        def cham_half(c, h, D16c):
            lhsT = T_L[:, c * 128:(c + 1) * 128]
            ch = dist.tile([128, 2048], F32, tag="d")
            for q in range(4):
                j0 = h * 2048 + q * 512
                nc.tensor.matmul(ch[:, q * 512:(q + 1) * 512], lhsT,
                                 T_RO[:, j0:j0 + 512], start=True, stop=True)
            nc.scalar.copy(D16c[:, h * 2048:(h + 1) * 2048], ch[:])

        dve_knn(0, u16_0)
        D16s = {}
        prevD = d16p.tile([128, npts], BF16, tag="D16c")
        for c in range(1, nch):
            u16 = small.tile([128, 16], F32, tag="u16")
            knn_half(c, 0, u16)
            knn_half(c, 1, u16)
            cham_half(c - 1, 0, prevD)
            cham_half(c - 1, 1, prevD)
            D16s[c - 1] = prevD
            prevD = d16p.tile([128, npts], BF16, tag="D16c")
            dve_knn(c, u16)
            if c >= 2:
                dve_ttr(c - 2, D16s.pop(c - 2))
        cham_half(nch - 1, 0, prevD)
        cham_half(nch - 1, 1, prevD)
        dve_ttr(nch - 2, D16s.pop(nch - 2))
        dve_ttr(nch - 1, prevD)

    # ---------------- finalize: per-batch scalars ----------------
    ones = singles.tile([128, 1], F32, tag="ones")
    nc.vector.memset(ones[:], 1.0)

    D = acc.tile([128, nch], F32, tag="D")     # chamfer min distances
    nc.vector.tensor_scalar_mul(D[:], MU[:], -1.0)
    VAL = acc.tile([128, nch], F32, tag="VAL")  # knn value_i
    nc.vector.tensor_scalar_mul(VAL[:], S5[:], -1.0 / K_NN)
    V2 = acc.tile([128, nch], F32, tag="V2")
    nc.vector.tensor_mul(V2[:], VAL[:], VAL[:])

    n = float(npts)
    st = small.tile([1, 12], F32, tag="st")
    outsb = small.tile([1, 2], F32, tag="outsb")
    with tc.tile_pool(name="cspsum", bufs=1, space="PSUM") as csp:
        cs = csp.tile([1, 3 * nch], F32, tag="cs")
        nc.tensor.matmul(cs[:, 0:nch], ones[:], D[:], start=True, stop=True)
        nc.tensor.matmul(cs[:, nch:2 * nch], ones[:], VAL[:], start=True, stop=True)
        nc.tensor.matmul(cs[:, 2 * nch:3 * nch], ones[:], V2[:], start=True, stop=True)
        nc.vector.tensor_reduce(
            st[:, 0:3], cs[:].rearrange("o (g x) -> o g x", g=3),
            axis=AX.X, op=ALU.add,
        )

        # chamfer_b
        nc.vector.tensor_scalar_mul(outsb[:, 0:1], st[:, 0:1], 1.0 / n)
        # value stats: mean, var (ddof=1), threshold
        nc.vector.tensor_scalar_mul(st[:, 3:4], st[:, 1:2], 1.0 / n)          # mean
        nc.vector.tensor_mul(st[:, 4:5], st[:, 1:2], st[:, 1:2])              # sumV^2
        nc.vector.tensor_scalar_mul(st[:, 5:6], st[:, 4:5], 1.0 / n)
        nc.vector.tensor_sub(st[:, 6:7], st[:, 2:3], st[:, 5:6])
        nc.vector.tensor_scalar_mul(st[:, 7:8], st[:, 6:7], 1.0 / (n - 1.0))  # var
        nc.scalar.sqrt(st[:, 8:9], st[:, 7:8])                                # std
        nc.vector.tensor_scalar_mul(st[:, 9:10], st[:, 8:9], ALPHA)
        nc.vector.tensor_add(st[:, 10:11], st[:, 3:4], st[:, 9:10])           # thr

        thrb = small.tile([128, 1], F32, tag="thrb")
        nc.gpsimd.partition_broadcast(thrb[:], st[:, 10:11])
        G = acc.tile([128, nch], F32, tag="G")
        nc.vector.tensor_scalar(G[:], VAL[:], thrb[:, 0:1], None, op0=ALU.is_gt)
        VM = acc.tile([128, nch], F32, tag="VM")
        nc.vector.tensor_mul(VM[:], VAL[:], G[:])
        cs2 = csp.tile([1, nch], F32, tag="cs2")
        nc.tensor.matmul(cs2[:, 0:nch], ones[:], VM[:], start=True, stop=True)
        nc.vector.tensor_reduce(st[:, 11:12], cs2[:, 0:nch], axis=AX.X, op=ALU.add)
        nc.vector.tensor_scalar_mul(outsb[:, 1:2], st[:, 11:12], 1.0 / n)

    nc.sync.dma_start(out=out[0:1, 0:2], in_=outsb[:])


def build_nc(npts=NPTS):
    nc = bacc.Bacc("TRN2", target_bir_lowering=False, debug=False)
    adv = nc.dram_tensor("adv", [npts, 3], F32, kind="ExternalInput")
    ori = nc.dram_tensor("ori", [npts, 3], F32, kind="ExternalInput")
    out = nc.dram_tensor("out", [1, 2], F32, kind="ExternalOutput")
    with tile.TileContext(nc) as tc, ExitStack() as ctx:
        build_body(tc, ctx, adv.ap(), ori.ap(), out.ap(), npts)
    nc.compile()
    return nc


_NC_CACHE = {}


def _get_nc(npts=NPTS):
    if npts not in _NC_CACHE:
        _NC_CACHE[npts] = build_nc(npts)
    return _NC_CACHE[npts]


def kernel(**inputs) -> np.ndarray:
    from concourse.bass_utils import run_bass_kernel_spmd

    adv = np.ascontiguousarray(np.asarray(inputs["adv_pc"], dtype=np.float32))
    ori = np.ascontiguousarray(np.asarray(inputs["ori_pc"], dtype=np.float32))
    B = adv.shape[0]
    assert B == N_CORES and adv.shape[1] == NPTS, (adv.shape, ori.shape)

    nc = _get_nc()
    in_maps = [{"adv": adv[b], "ori": ori[b]} for b in range(B)]
    res = run_bass_kernel_spmd(nc, in_maps, core_ids=list(range(N_CORES)))
    parts = np.stack([r["out"][0] for r in res.results])  # [B, 2]
    loss = W_CHAMFER * parts[:, 0].mean() + W_KNN * parts[:, 1].mean()
    return np.float32(loss)


# revision 38
# speedup vs baseline: 1.0193x; 1.0024x over previous
"""ChamferkNNDist kernel for Trainium2 (8 NeuronCores, pure data parallel).

Reference math (per batch element b, K=4096 points, 3 dims):
  chamfer_b = mean_i min_j ||adv_i - ori_j||^2
  dd_ij     = ||adv_i - adv_j||^2
  value_i   = mean of the 5 smallest dd_ij excluding self
  knn_b     = mean_i value_i * [value_i > mean(value) + 1.05*std(value, ddof=1)]
  loss      = 5 * mean_b chamfer_b + 3 * mean_b knn_b

Device strategy (one batch element per core):
  The PE emits NEGATED squared distances directly: the 13-row bf16
  contraction computes -d_ij = 2 a_i.b_j - |b_j|^2 - |a_i|^2 with every
  fp32 factor compensated-split into bf16 hi+lo (dropped lo*lo cross terms
  leave ~1e-4 abs error; the row-constant |a_i|^2 rides along as two extra
  lhsT rows against all-ones rhs rows, so the cancellation happens in fp32
  PSUM). Row pairing (lhsT x rhs), with A = 2a:
    k0-2: Ah.bh   k3-5: Al.bh   k6-8: Ah.bl   k9,10: 1*(-bb hi,lo)
    k11,12: (-aa hi,lo)*1
  Streaming cost is 1 column/cycle regardless of the 13 rows.

  Per 128-query chunk, PSUM holds -d in two f32 [128,2048] halves.
  Real-hardware constraints discovered by probing the walrus verifier and
  the device: GPSIMD cannot touch PSUM or run max ALU ops, DMA cannot read
  PSUM, matmul output must be f32, and tensor_tensor_reduce faults at
  runtime -- so DVE carries all compare work:
  - kNN: DVE max8 directly on each PSUM half (top-8 of -d; rank 1 = self
    at ~0), a [128,16] merge max8, value_i = -mean(ranks 2..6).
  - chamfer: ACT drains halves to a bf16 [128,4096] row (relative
    precision is preserved because -d is small near the min); DVE runs a
    bf16 2x-mode pairwise-max tree + small reduce, emitted two chunks
    late so it never gates the max8s. D = -max(-d).
  Chamfer matmuls trail the kNN matmuls by one chunk in the PE stream,
  and a PE warm-up stream ramps the clock before the transposes.
  Batch stats (mean/std/threshold/masked mean) on device via ones-matmul
  column sums; host only averages the 8 per-core (chamfer_b, knn_b) pairs.
"""

import os
import sys
from contextlib import ExitStack

import numpy as np

try:
    import concourse  # noqa: F401
except ImportError:  # staged repo location inside the container
    for _p in ("/opt/trn_rl_repo", os.path.expanduser("~/.axon_site/_ro/trn_rl_repo")):
        if os.path.isdir(_p):
            sys.path.insert(0, _p)
            break

import concourse.bacc as bacc
import concourse.tile as tile
from concourse import mybir

F32 = mybir.dt.float32
BF16 = mybir.dt.bfloat16
ALU = mybir.AluOpType
AX = mybir.AxisListType

NPTS = 4096
N_CORES = 8
K_NN = 5
ALPHA = 1.05
W_CHAMFER = 5.0
W_KNN = 3.0
NEG_INF = -3.0e38
NROW = 13


def build_body(tc, ctx: ExitStack, adv, ori, out, npts):
    nc = tc.nc
    nch = npts // 128

    singles = ctx.enter_context(tc.tile_pool(name="singles", bufs=1))
    prep = ctx.enter_context(tc.tile_pool(name="prep", bufs=1))
    feat = ctx.enter_context(tc.tile_pool(name="feat", bufs=1))
    acc = ctx.enter_context(tc.tile_pool(name="acc", bufs=1))
    d16p = ctx.enter_context(tc.tile_pool(name="d16p", bufs=3))
    champ = ctx.enter_context(tc.tile_pool(name="champ", bufs=3))
    scrp = ctx.enter_context(tc.tile_pool(name="scrp", bufs=3))
    small = ctx.enter_context(tc.tile_pool(name="small", bufs=3))
    d16kp = ctx.enter_context(tc.tile_pool(name="d16kp", bufs=2))

    # ---------------- identity + PE warm-up ----------------
    # The PE clock ramps with sustained use; stream throwaway matmuls while
    # the DMA + staging prep runs so the transposes and first chunks start
    # at full speed.
    ident_i = singles.tile([128, 128], mybir.dt.int32, tag="identI")
    nc.gpsimd.iota(ident_i[:], pattern=[[1, 128]], base=0, channel_multiplier=-1)
    ident = singles.tile([128, 128], BF16, tag="ident")
    nc.vector.tensor_scalar(ident[:], ident_i[:], 0.0, None, op0=ALU.is_equal)
    wrm = singles.tile([128, 512], BF16, tag="wrm")
    nc.gpsimd.memset(wrm[:], 0.5)
    with tc.tile_pool(name="wpsum", bufs=1, space="PSUM") as wpsum:
        wps = wpsum.tile([128, 512], F32, tag="wps")
        for _ in range(12):
            nc.tensor.matmul(wps[:], wrm[:, 0:128], wrm[:], start=True, stop=True)

    # ---------------- load points (contiguous; point order is a
    # permutation, and every reduction here is permutation-invariant) ------
    P_a = prep.tile([128, nch, 3], F32, tag="P_a")
    nc.sync.dma_start(out=P_a[:], in_=adv.rearrange("(p c) d -> p c d", c=nch))
    P_o = prep.tile([128, nch, 3], F32, tag="P_o")
    nc.sync.dma_start(out=P_o[:], in_=ori.rearrange("(p c) d -> p c d", c=nch))

    # ---------------- negated squared norms ----------------
    def norms(P, tag):
        sq = prep.tile([128, nch, 3], F32, tag=f"sq{tag}")
        nc.vector.tensor_mul(sq[:], P[:], P[:])
        nn = prep.tile([128, nch, 1], F32, tag=f"nn{tag}")
        nc.vector.tensor_reduce(nn[:], sq[:], axis=AX.X, op=ALU.add)
        ng = prep.tile([128, nch, 1], F32, tag=f"ng{tag}")
        nc.vector.tensor_scalar_mul(ng[:], nn[:], -1.0)
        return nn, ng

    aa, naa = norms(P_a, "a")   # aa = |a|^2,  naa = -aa
    _bb, nbb = norms(P_o, "o")

    # naa bf16 hi/lo split, shared by S_L rows 11,12 and S_RA rows 9,10
    sh3 = [128, nch, 3]
    sh1 = [128, nch, 1]
    nah = prep.tile(sh1, BF16, tag="nah")
    nal = prep.tile(sh1, BF16, tag="nal")
    nc.scalar.copy(nah[:], naa[:])
    r0 = prep.tile(sh1, F32, tag="r0")
    nc.vector.tensor_sub(r0[:], naa[:], nah[:])
    nc.scalar.copy(nal[:], r0[:])

    # ---------------- bf16 hi/lo staging, point-major [128, nch, 13] ------
    S_L = prep.tile([128, nch, NROW], BF16, tag="S_L")
    # lhsT rows: Ah(0:3), Al(3:6), Ah dup(6:9), 1(9:11), nah(11), nal(12)
    B2 = prep.tile(sh3, F32, tag="B2")
    nc.vector.tensor_scalar_mul(B2[:], P_a[:], 2.0)
    nc.scalar.copy(S_L[:, :, 0:3], B2[:])                     # Ah = bf16(2a)
    rl = prep.tile(sh3, F32, tag="rl")
    nc.vector.tensor_sub(rl[:], B2[:], S_L[:, :, 0:3])
    nc.scalar.copy(S_L[:, :, 3:6], rl[:])                     # Al
    nc.vector.tensor_copy(S_L[:, :, 6:9], S_L[:, :, 0:3])
    nc.gpsimd.memset(S_L[:, :, 9:11], 1.0)
    nc.vector.tensor_copy(S_L[:, :, 11:12], nah[:])
    nc.vector.tensor_copy(S_L[:, :, 12:13], nal[:])

    def build_rhs(P, nh_src, nl_src, ng, tag, eng, cast):
        # rhs rows: bh(0:3), bh dup(3:6), bl(6:9), nb hi(9), nb lo(10),
        # ones(11:13). Chain on one engine so the two rhs builds overlap.
        S = prep.tile([128, nch, NROW], BF16, tag=f"S_{tag}")
        cast(S[:, :, 0:3], P[:])                              # bh
        r2 = prep.tile(sh3, F32, tag=f"r2_{tag}")
        eng.tensor_sub(r2[:], P[:], S[:, :, 0:3])
        cast(S[:, :, 6:9], r2[:])                             # bl
        eng.tensor_copy(S[:, :, 3:6], S[:, :, 0:3])
        if nh_src is not None:
            eng.tensor_copy(S[:, :, 9:10], nh_src[:])
            eng.tensor_copy(S[:, :, 10:11], nl_src[:])
        else:
            cast(S[:, :, 9:10], ng[:])                        # nb hi
            r3 = prep.tile(sh1, F32, tag=f"r3_{tag}")
            eng.tensor_sub(r3[:], ng[:], S[:, :, 9:10])
            cast(S[:, :, 10:11], r3[:])                       # nb lo
        nc.gpsimd.memset(S[:, :, 11:13], 1.0)
        return S

    S_RA = build_rhs(P_a, nah, nal, None, "ra", nc.gpsimd,
                     lambda o, i: nc.gpsimd.tensor_copy(o, i))
    S_RO = build_rhs(P_o, None, None, nbb, "ro", nc.vector,
                     lambda o, i: nc.vector.tensor_copy(o, i))

    # ---------------- transpose staging -> feature-major [13, npts] -------
    T_L = feat.tile([NROW, npts], BF16, tag="T_L")
    T_RA = feat.tile([NROW, npts], BF16, tag="T_RA")
    T_RO = feat.tile([NROW, npts], BF16, tag="T_RO")

    S5 = acc.tile([128, nch], F32, tag="S5")   # sum of -d ranks 2..6 (knn)
    MU = acc.tile([128, nch], F32, tag="MU")   # chamfer row max of -d

    act_cp = lambda o, i: nc.scalar.copy(o, i)        # noqa: E731
    dve_cp = lambda o, i: nc.vector.tensor_copy(o, i)  # noqa: E731

    # knn: two max8 straight off the f32 -d PSUM halves (no drain).
    # cham: ACT drains halves to a bf16 [128,4096] row; one DVE ttr
    # (trailing two chunks) reduces it. Pool/GPSIMD cannot max on real HW,
    # and ttr/DMA cannot touch PSUM, so DVE carries all compare work.
    with tc.tile_pool(name="tpsum", bufs=4, space="PSUM") as tpsum, \
         tc.tile_pool(name="k0dist", bufs=1, space="PSUM") as k0dist:

        def tgroup(S, T, g, drain):
            pt = tpsum.tile([NROW, 1024], BF16, tag="pt")
            for ci in range(8):
                c = g * 8 + ci
                nc.tensor.transpose(
                    pt[:, ci * 128:(ci + 1) * 128], S[:, c, :], ident[:]
                )
            drain(T[:, g * 1024:(g + 1) * 1024], pt[:])

        for g in range(4):
            tgroup(S_L, T_L, g, dve_cp)
        for g in range(4):
            tgroup(S_RA, T_RA, g, act_cp if g < 2 else dve_cp)

        # chunk 0 kNN via a single-buffered tile so it overlaps the T_RO
        # transposes (the main dist pool needs all 8 banks)
        dk0 = d16kp.tile([128, 2056], F32, tag="D16k")
        lhsT0 = T_L[:, 0:128]
        for h in range(2):
            kh = k0dist.tile([128, 2048], F32, tag="k0")
            for q in range(4):
                j0 = h * 2048 + q * 512
                nc.tensor.matmul(kh[:, q * 512:(q + 1) * 512], lhsT0,
                                 T_RA[:, j0:j0 + 512], start=True, stop=True)
            if h == 0:
                # H0's top-8 lands in the tail of the drain tile, so one
                # max8 over [128,2056] gives the global top-8 (no merge)
                nc.vector.max(out=dk0[:, 2048:2056], in_=kh[:])
            else:
                nc.scalar.copy(dk0[:, 0:2048], kh[:])
        for g in range(4):
            tgroup(S_RO, T_RO, g, act_cp)

    def dve_knn(c, dk):
        u8 = small.tile([128, 8], F32, tag="u8")
        nc.vector.max(out=u8[:], in_=dk[:])
        nc.vector.tensor_reduce(S5[:, c:c + 1], u8[:, 1:6], axis=AX.X,
                                op=ALU.add)

    def dve_ttr(c, D16c):
        # emitted two chunks late so DVE never stalls on ACT's drains.
        # tensor_tensor_reduce faults on real trn2, so this is a bf16
        # 2x-mode pairwise-max tree (the instruction mix the baseline
        # proved on hardware) plus one small reduce.
        t1 = scrp.tile([128, 2048], BF16, tag="t1")
        nc.vector.tensor_tensor(t1[:], D16c[:, 0:2048], D16c[:, 2048:4096],
                                op=ALU.max)
        t2 = scrp.tile([128, 1024], BF16, tag="t2")
        nc.vector.tensor_tensor(t2[:], t1[:, 0:1024], t1[:, 1024:2048],
                                op=ALU.max)
        t3 = scrp.tile([128, 512], BF16, tag="t3")
        nc.vector.tensor_tensor(t3[:], t2[:, 0:512], t2[:, 512:1024],
                                op=ALU.max)
        t4 = scrp.tile([128, 256], BF16, tag="t4")
        nc.vector.tensor_tensor(t4[:], t3[:, 0:256], t3[:, 256:512],
                                op=ALU.max)
        nc.vector.tensor_reduce(MU[:, c:c + 1], t4[:], axis=AX.X, op=ALU.max)

    with tc.tile_pool(name="dist", bufs=2, space="PSUM") as dist:

        def knn_half(c, h, dk):
            # H0: max8 straight off PSUM, its top-8 written into the tail
            # of the H1 drain tile. H1: ACT (which has slack) drains to
            # SBUF f32; dve_knn's single max8 over [128,2056] then yields
            # the global top-8 with no separate merge.
            lhsT = T_L[:, c * 128:(c + 1) * 128]
            kh = dist.tile([128, 2048], F32, tag="d")
            for q in range(4):
                j0 = h * 2048 + q * 512
                nc.tensor.matmul(kh[:, q * 512:(q + 1) * 512], lhsT,
                                 T_RA[:, j0:j0 + 512], start=True, stop=True)
            if h == 0:
                nc.vector.max(out=dk[:, 2048:2056], in_=kh[:])
            else:
                nc.scalar.copy(dk[:, 0:2048], kh[:])

        def cham_half(c, h, D16c):
            lhsT = T_L[:, c * 128:(c + 1) * 128]
            ch = dist.tile([128, 2048], F32, tag="d")
            for q in range(4):
                j0 = h * 2048 + q * 512
                nc.tensor.matmul(ch[:, q * 512:(q + 1) * 512], lhsT,
                                 T_RO[:, j0:j0 + 512], start=True, stop=True)
            nc.scalar.copy(D16c[:, h * 2048:(h + 1) * 2048], ch[:])

        dve_knn(0, dk0)
        D16s = {}
        prevD = d16p.tile([128, npts], BF16, tag="D16c")
        for c in range(1, nch):
            dk = d16kp.tile([128, 2056], F32, tag="D16k")
            knn_half(c, 0, dk)
            knn_half(c, 1, dk)
            cham_half(c - 1, 0, prevD)
            cham_half(c - 1, 1, prevD)
            D16s[c - 1] = prevD
            prevD = d16p.tile([128, npts], BF16, tag="D16c")
            dve_knn(c, dk)
            if c >= 2:
                dve_ttr(c - 2, D16s.pop(c - 2))
        cham_half(nch - 1, 0, prevD)
        cham_half(nch - 1, 1, prevD)
        dve_ttr(nch - 2, D16s.pop(nch - 2))
        dve_ttr(nch - 1, prevD)

    # ---------------- finalize: per-batch scalars ----------------
    ones = singles.tile([128, 1], F32, tag="ones")
    nc.vector.memset(ones[:], 1.0)

    D = acc.tile([128, nch], F32, tag="D")     # chamfer min distances
    nc.vector.tensor_scalar_mul(D[:], MU[:], -1.0)
    VAL = acc.tile([128, nch], F32, tag="VAL")  # knn value_i
    nc.vector.tensor_scalar_mul(VAL[:], S5[:], -1.0 / K_NN)
    V2 = acc.tile([128, nch], F32, tag="V2")
    nc.vector.tensor_mul(V2[:], VAL[:], VAL[:])

    n = float(npts)
    st = small.tile([1, 12], F32, tag="st")
    outsb = small.tile([1, 2], F32, tag="outsb")
    with tc.tile_pool(name="cspsum", bufs=1, space="PSUM") as csp:
        cs = csp.tile([1, 3 * nch], F32, tag="cs")
        nc.tensor.matmul(cs[:, 0:nch], ones[:], D[:], start=True, stop=True)
        nc.tensor.matmul(cs[:, nch:2 * nch], ones[:], VAL[:], start=True, stop=True)
        nc.tensor.matmul(cs[:, 2 * nch:3 * nch], ones[:], V2[:], start=True, stop=True)
        nc.vector.tensor_reduce(
            st[:, 0:3], cs[:].rearrange("o (g x) -> o g x", g=3),
            axis=AX.X, op=ALU.add,
        )

        # chamfer_b
        nc.vector.tensor_scalar_mul(outsb[:, 0:1], st[:, 0:1], 1.0 / n)
        # value stats: mean, var (ddof=1), threshold
        nc.vector.tensor_scalar_mul(st[:, 3:4], st[:, 1:2], 1.0 / n)          # mean
        nc.vector.tensor_mul(st[:, 4:5], st[:, 1:2], st[:, 1:2])              # sumV^2
        nc.vector.tensor_scalar_mul(st[:, 5:6], st[:, 4:5], 1.0 / n)
        nc.vector.tensor_sub(st[:, 6:7], st[:, 2:3], st[:, 5:6])
        nc.vector.tensor_scalar_mul(st[:, 7:8], st[:, 6:7], 1.0 / (n - 1.0))  # var
        nc.scalar.sqrt(st[:, 8:9], st[:, 7:8])                                # std
        nc.vector.tensor_scalar_mul(st[:, 9:10], st[:, 8:9], ALPHA)
        nc.vector.tensor_add(st[:, 10:11], st[:, 3:4], st[:, 9:10])           # thr

        thrb = small.tile([128, 1], F32, tag="thrb")
        nc.gpsimd.partition_broadcast(thrb[:], st[:, 10:11])
        G = acc.tile([128, nch], F32, tag="G")
        nc.vector.tensor_scalar(G[:], VAL[:], thrb[:, 0:1], None, op0=ALU.is_gt)
        VM = acc.tile([128, nch], F32, tag="VM")
        nc.vector.tensor_mul(VM[:], VAL[:], G[:])
        cs2 = csp.tile([1, nch], F32, tag="cs2")
        nc.tensor.matmul(cs2[:, 0:nch], ones[:], VM[:], start=True, stop=True)
        nc.vector.tensor_reduce(st[:, 11:12], cs2[:, 0:nch], axis=AX.X, op=ALU.add)
        nc.vector.tensor_scalar_mul(outsb[:, 1:2], st[:, 11:12], 1.0 / n)

    nc.sync.dma_start(out=out[0:1, 0:2], in_=outsb[:])


def build_nc(npts=NPTS):
    nc = bacc.Bacc("TRN2", target_bir_lowering=False, debug=False)
    adv = nc.dram_tensor("adv", [npts, 3], F32, kind="ExternalInput")
    ori = nc.dram_tensor("ori", [npts, 3], F32, kind="ExternalInput")
    out = nc.dram_tensor("out", [1, 2], F32, kind="ExternalOutput")
    with tile.TileContext(nc) as tc, ExitStack() as ctx:
        build_body(tc, ctx, adv.ap(), ori.ap(), out.ap(), npts)
    nc.compile()
    return nc


_NC_CACHE = {}


def _get_nc(npts=NPTS):
    if npts not in _NC_CACHE:
        _NC_CACHE[npts] = build_nc(npts)
    return _NC_CACHE[npts]


def kernel(**inputs) -> np.ndarray:
    from concourse.bass_utils import run_bass_kernel_spmd

    adv = np.ascontiguousarray(np.asarray(inputs["adv_pc"], dtype=np.float32))
    ori = np.ascontiguousarray(np.asarray(inputs["ori_pc"], dtype=np.float32))
    B = adv.shape[0]
    assert B == N_CORES and adv.shape[1] == NPTS, (adv.shape, ori.shape)

    nc = _get_nc()
    in_maps = [{"adv": adv[b], "ori": ori[b]} for b in range(B)]
    res = run_bass_kernel_spmd(nc, in_maps, core_ids=list(range(N_CORES)))
    parts = np.stack([r["out"][0] for r in res.results])  # [B, 2]
    loss = W_CHAMFER * parts[:, 0].mean() + W_KNN * parts[:, 1].mean()
    return np.float32(loss)


# revision 40
# speedup vs baseline: 1.0242x; 1.0048x over previous
"""ChamferkNNDist kernel for Trainium2 (8 NeuronCores, pure data parallel).

Reference math (per batch element b, K=4096 points, 3 dims):
  chamfer_b = mean_i min_j ||adv_i - ori_j||^2
  dd_ij     = ||adv_i - adv_j||^2
  value_i   = mean of the 5 smallest dd_ij excluding self
  knn_b     = mean_i value_i * [value_i > mean(value) + 1.05*std(value, ddof=1)]
  loss      = 5 * mean_b chamfer_b + 3 * mean_b knn_b

Device strategy (one batch element per core):
  The PE emits NEGATED squared distances directly: the 13-row bf16
  contraction computes -d_ij = 2 a_i.b_j - |b_j|^2 - |a_i|^2 with every
  fp32 factor compensated-split into bf16 hi+lo (dropped lo*lo cross terms
  leave ~1e-4 abs error; the row-constant |a_i|^2 rides along as two extra
  lhsT rows against all-ones rhs rows, so the cancellation happens in fp32
  PSUM). Row pairing (lhsT x rhs), with A = 2a:
    k0-2: Ah.bh   k3-5: Al.bh   k6-8: Ah.bl   k9,10: 1*(-bb hi,lo)
    k11,12: (-aa hi,lo)*1
  Streaming cost is 1 column/cycle regardless of the 13 rows.

  Per 128-query chunk, PSUM holds -d in two f32 [128,2048] halves.
  Real-hardware constraints discovered by probing the walrus verifier and
  the device: GPSIMD cannot touch PSUM or run max ALU ops, DMA cannot read
  PSUM, matmul output must be f32, and tensor_tensor_reduce faults at
  runtime -- so DVE carries all compare work:
  - kNN: DVE max8 directly on each PSUM half (top-8 of -d; rank 1 = self
    at ~0), a [128,16] merge max8, value_i = -mean(ranks 2..6).
  - chamfer: ACT drains halves to a bf16 [128,4096] row (relative
    precision is preserved because -d is small near the min); DVE runs a
    bf16 2x-mode pairwise-max tree + small reduce, emitted two chunks
    late so it never gates the max8s. D = -max(-d).
  Chamfer matmuls trail the kNN matmuls by one chunk in the PE stream,
  and a PE warm-up stream ramps the clock before the transposes.
  Batch stats (mean/std/threshold/masked mean) on device via ones-matmul
  column sums; host only averages the 8 per-core (chamfer_b, knn_b) pairs.
"""

import os
import sys
from contextlib import ExitStack

import numpy as np

try:
    import concourse  # noqa: F401
except ImportError:  # staged repo location inside the container
    for _p in ("/opt/trn_rl_repo", os.path.expanduser("~/.axon_site/_ro/trn_rl_repo")):
        if os.path.isdir(_p):
            sys.path.insert(0, _p)
            break

import concourse.bacc as bacc
import concourse.tile as tile
from concourse import mybir

F32 = mybir.dt.float32
BF16 = mybir.dt.bfloat16
ALU = mybir.AluOpType
AX = mybir.AxisListType

NPTS = 4096
N_CORES = 8
K_NN = 5
ALPHA = 1.05
W_CHAMFER = 5.0
W_KNN = 3.0
NEG_INF = -3.0e38
NROW = 13


def build_body(tc, ctx: ExitStack, adv, ori, out, npts):
    nc = tc.nc
    nch = npts // 128

    singles = ctx.enter_context(tc.tile_pool(name="singles", bufs=1))
    prep = ctx.enter_context(tc.tile_pool(name="prep", bufs=1))
    feat = ctx.enter_context(tc.tile_pool(name="feat", bufs=1))
    acc = ctx.enter_context(tc.tile_pool(name="acc", bufs=1))
    d16p = ctx.enter_context(tc.tile_pool(name="d16p", bufs=3))
    champ = ctx.enter_context(tc.tile_pool(name="champ", bufs=3))
    scrp = ctx.enter_context(tc.tile_pool(name="scrp", bufs=3))
    small = ctx.enter_context(tc.tile_pool(name="small", bufs=3))
    d16kp = ctx.enter_context(tc.tile_pool(name="d16kp", bufs=2))

    # ---------------- identity + PE warm-up ----------------
    # The PE clock ramps with sustained use; stream throwaway matmuls while
    # the DMA + staging prep runs so the transposes and first chunks start
    # at full speed.
    ident_i = singles.tile([128, 128], mybir.dt.int32, tag="identI")
    nc.gpsimd.iota(ident_i[:], pattern=[[1, 128]], base=0, channel_multiplier=-1)
    ident = singles.tile([128, 128], BF16, tag="ident")
    nc.vector.tensor_scalar(ident[:], ident_i[:], 0.0, None, op0=ALU.is_equal)
    wrm = singles.tile([128, 512], BF16, tag="wrm")
    nc.gpsimd.memset(wrm[:], 0.5)
    with tc.tile_pool(name="wpsum", bufs=1, space="PSUM") as wpsum:
        wps = wpsum.tile([128, 512], F32, tag="wps")
        for _ in range(12):
            nc.tensor.matmul(wps[:], wrm[:, 0:128], wrm[:], start=True, stop=True)

    # ---------------- load points (contiguous; point order is a
    # permutation, and every reduction here is permutation-invariant) ------
    P_a = prep.tile([128, nch, 3], F32, tag="P_a")
    nc.sync.dma_start(out=P_a[:], in_=adv.rearrange("(p c) d -> p c d", c=nch))
    P_o = prep.tile([128, nch, 3], F32, tag="P_o")
    nc.sync.dma_start(out=P_o[:], in_=ori.rearrange("(p c) d -> p c d", c=nch))

    # ---------------- negated squared norms ----------------
    def norms(P, tag):
        sq = prep.tile([128, nch, 3], F32, tag=f"sq{tag}")
        nc.vector.tensor_mul(sq[:], P[:], P[:])
        nn = prep.tile([128, nch, 1], F32, tag=f"nn{tag}")
        nc.vector.tensor_reduce(nn[:], sq[:], axis=AX.X, op=ALU.add)
        ng = prep.tile([128, nch, 1], F32, tag=f"ng{tag}")
        nc.vector.tensor_scalar_mul(ng[:], nn[:], -1.0)
        return nn, ng

    aa, naa = norms(P_a, "a")   # aa = |a|^2,  naa = -aa
    _bb, nbb = norms(P_o, "o")

    # naa bf16 hi/lo split, shared by S_L rows 11,12 and S_RA rows 9,10
    sh3 = [128, nch, 3]
    sh1 = [128, nch, 1]
    nah = prep.tile(sh1, BF16, tag="nah")
    nal = prep.tile(sh1, BF16, tag="nal")
    nc.scalar.copy(nah[:], naa[:])
    r0 = prep.tile(sh1, F32, tag="r0")
    nc.vector.tensor_sub(r0[:], naa[:], nah[:])
    nc.scalar.copy(nal[:], r0[:])

    # ---------------- bf16 hi/lo staging, point-major [128, nch, 13] ------
    S_L = prep.tile([128, nch, NROW], BF16, tag="S_L")
    # lhsT rows: Ah(0:3), Al(3:6), Ah dup(6:9), 1(9:11), nah(11), nal(12)
    B2 = prep.tile(sh3, F32, tag="B2")
    nc.vector.tensor_scalar_mul(B2[:], P_a[:], 2.0)
    nc.scalar.copy(S_L[:, :, 0:3], B2[:])                     # Ah = bf16(2a)
    rl = prep.tile(sh3, F32, tag="rl")
    nc.vector.tensor_sub(rl[:], B2[:], S_L[:, :, 0:3])
    nc.scalar.copy(S_L[:, :, 3:6], rl[:])                     # Al
    nc.vector.tensor_copy(S_L[:, :, 6:9], S_L[:, :, 0:3])
    nc.gpsimd.memset(S_L[:, :, 9:11], 1.0)
    nc.vector.tensor_copy(S_L[:, :, 11:12], nah[:])
    nc.vector.tensor_copy(S_L[:, :, 12:13], nal[:])

    def build_rhs(P, nh_src, nl_src, ng, tag, eng, cast):
        # rhs rows: bh(0:3), bh dup(3:6), bl(6:9), nb hi(9), nb lo(10),
        # ones(11:13). Chain on one engine so the two rhs builds overlap.
        S = prep.tile([128, nch, NROW], BF16, tag=f"S_{tag}")
        cast(S[:, :, 0:3], P[:])                              # bh
        r2 = prep.tile(sh3, F32, tag=f"r2_{tag}")
        eng.tensor_sub(r2[:], P[:], S[:, :, 0:3])
        cast(S[:, :, 6:9], r2[:])                             # bl
        eng.tensor_copy(S[:, :, 3:6], S[:, :, 0:3])
        if nh_src is not None:
            eng.tensor_copy(S[:, :, 9:10], nh_src[:])
            eng.tensor_copy(S[:, :, 10:11], nl_src[:])
        else:
            cast(S[:, :, 9:10], ng[:])                        # nb hi
            r3 = prep.tile(sh1, F32, tag=f"r3_{tag}")
            eng.tensor_sub(r3[:], ng[:], S[:, :, 9:10])
            cast(S[:, :, 10:11], r3[:])                       # nb lo
        nc.gpsimd.memset(S[:, :, 11:13], 1.0)
        return S

    S_RA = build_rhs(P_a, nah, nal, None, "ra", nc.gpsimd,
                     lambda o, i: nc.gpsimd.tensor_copy(o, i))
    S_RO = build_rhs(P_o, None, None, nbb, "ro", nc.vector,
                     lambda o, i: nc.vector.tensor_copy(o, i))

    # ---------------- transpose staging -> feature-major [13, npts] -------
    T_L = feat.tile([NROW, npts], BF16, tag="T_L")
    T_RA = feat.tile([NROW, npts], BF16, tag="T_RA")
    T_RO = feat.tile([NROW, npts], BF16, tag="T_RO")

    S5 = acc.tile([128, nch], F32, tag="S5")   # sum of -d ranks 2..6 (knn)
    MU = acc.tile([128, nch], F32, tag="MU")   # chamfer row max of -d

    act_cp = lambda o, i: nc.scalar.copy(o, i)        # noqa: E731
    dve_cp = lambda o, i: nc.vector.tensor_copy(o, i)  # noqa: E731

    # knn: two max8 straight off the f32 -d PSUM halves (no drain).
    # cham: ACT drains halves to a bf16 [128,4096] row; one DVE ttr
    # (trailing two chunks) reduces it. Pool/GPSIMD cannot max on real HW,
    # and ttr/DMA cannot touch PSUM, so DVE carries all compare work.
    with tc.tile_pool(name="tpsum", bufs=4, space="PSUM") as tpsum, \
         tc.tile_pool(name="k0dist", bufs=1, space="PSUM") as k0dist:

        def tgroup(S, T, g, drain):
            pt = tpsum.tile([NROW, 1024], BF16, tag="pt")
            for ci in range(8):
                c = g * 8 + ci
                nc.tensor.transpose(
                    pt[:, ci * 128:(ci + 1) * 128], S[:, c, :], ident[:]
                )
            drain(T[:, g * 1024:(g + 1) * 1024], pt[:])

        for g in range(4):
            tgroup(S_L, T_L, g, dve_cp)
        for g in range(4):
            tgroup(S_RA, T_RA, g, act_cp if g < 2 else dve_cp)

        # chunk 0 kNN via a single-buffered tile so it overlaps the T_RO
        # transposes (the main dist pool needs all 8 banks)
        dk0 = d16kp.tile([128, 2056], F32, tag="D16k")
        lhsT0 = T_L[:, 0:128]
        for h in range(2):
            kh = k0dist.tile([128, 2048], F32, tag="k0")
            for q in range(4):
                j0 = h * 2048 + q * 512
                nc.tensor.matmul(kh[:, q * 512:(q + 1) * 512], lhsT0,
                                 T_RA[:, j0:j0 + 512], start=True, stop=True)
            if h == 0:
                # H0's top-8 lands in the tail of the drain tile, so one
                # max8 over [128,2056] gives the global top-8 (no merge)
                nc.vector.max(out=dk0[:, 2048:2056], in_=kh[:])
            else:
                nc.scalar.copy(dk0[:, 0:2048], kh[:])
        for g in range(4):
            tgroup(S_RO, T_RO, g, act_cp)

    def dve_knn(c, dk):
        u8 = small.tile([128, 8], F32, tag="u8")
        nc.vector.max(out=u8[:], in_=dk[:])
        nc.vector.tensor_reduce(S5[:, c:c + 1], u8[:, 1:6], axis=AX.X,
                                op=ALU.add)

    # Chamfer reduce, emitted two chunks late so DVE never stalls on ACT's
    # drains. tensor_tensor_reduce faults on real trn2, so this is a bf16
    # 2x-mode pairwise-max tree; the two small tail levels + final reduce
    # are batched per chunk PAIR (3D APs, baseline-proven) to halve their
    # fixed per-op overheads.
    TP = {"t": None}

    def dve_ttr(c, D16c):
        b = c % 2
        if b == 0:
            tp_tile = scrp.tile([128, 2, 512], BF16, tag="TP")
            TP["t"] = tp_tile
        t1 = scrp.tile([128, 2048], BF16, tag="t1")
        nc.vector.tensor_tensor(t1[:], D16c[:, 0:2048], D16c[:, 2048:4096],
                                op=ALU.max)
        t2 = scrp.tile([128, 1024], BF16, tag="t2")
        nc.vector.tensor_tensor(t2[:], t1[:, 0:1024], t1[:, 1024:2048],
                                op=ALU.max)
        nc.vector.tensor_tensor(TP["t"][:, b, :], t2[:, 0:512], t2[:, 512:1024],
                                op=ALU.max)
        if b == 1:
            T = TP["t"]
            t4 = scrp.tile([128, 2, 256], BF16, tag="t4")
            nc.vector.tensor_tensor(t4[:], T[:, :, 0:256], T[:, :, 256:512],
                                    op=ALU.max)
            nc.vector.tensor_reduce(MU[:, c - 1:c + 1], t4[:], axis=AX.X,
                                    op=ALU.max)

    with tc.tile_pool(name="dist", bufs=2, space="PSUM") as dist:

        def knn_half(c, h, dk):
            # H0: max8 straight off PSUM, its top-8 written into the tail
            # of the H1 drain tile. H1: ACT (which has slack) drains to
            # SBUF f32; dve_knn's single max8 over [128,2056] then yields
            # the global top-8 with no separate merge.
            lhsT = T_L[:, c * 128:(c + 1) * 128]
            kh = dist.tile([128, 2048], F32, tag="d")
            for q in range(4):
                j0 = h * 2048 + q * 512
                nc.tensor.matmul(kh[:, q * 512:(q + 1) * 512], lhsT,
                                 T_RA[:, j0:j0 + 512], start=True, stop=True)
            if h == 0:
                nc.vector.max(out=dk[:, 2048:2056], in_=kh[:])
            else:
                nc.scalar.copy(dk[:, 0:2048], kh[:])

        def cham_half(c, h, D16c):
            lhsT = T_L[:, c * 128:(c + 1) * 128]
            ch = dist.tile([128, 2048], F32, tag="d")
            for q in range(4):
                j0 = h * 2048 + q * 512
                nc.tensor.matmul(ch[:, q * 512:(q + 1) * 512], lhsT,
                                 T_RO[:, j0:j0 + 512], start=True, stop=True)
            nc.scalar.copy(D16c[:, h * 2048:(h + 1) * 2048], ch[:])

        dve_knn(0, dk0)
        D16s = {}
        prevD = d16p.tile([128, npts], BF16, tag="D16c")
        for c in range(1, nch):
            dk = d16kp.tile([128, 2056], F32, tag="D16k")
            knn_half(c, 0, dk)
            knn_half(c, 1, dk)
            cham_half(c - 1, 0, prevD)
            cham_half(c - 1, 1, prevD)
            D16s[c - 1] = prevD
            prevD = d16p.tile([128, npts], BF16, tag="D16c")
            dve_knn(c, dk)
            if c >= 2:
                dve_ttr(c - 2, D16s.pop(c - 2))
        cham_half(nch - 1, 0, prevD)
        cham_half(nch - 1, 1, prevD)
        dve_ttr(nch - 2, D16s.pop(nch - 2))
        dve_ttr(nch - 1, prevD)

    # ---------------- finalize: per-batch scalars ----------------
    ones = singles.tile([128, 1], F32, tag="ones")
    nc.vector.memset(ones[:], 1.0)

    D = acc.tile([128, nch], F32, tag="D")     # chamfer min distances
    nc.vector.tensor_scalar_mul(D[:], MU[:], -1.0)
    VAL = acc.tile([128, nch], F32, tag="VAL")  # knn value_i
    nc.vector.tensor_scalar_mul(VAL[:], S5[:], -1.0 / K_NN)
    V2 = acc.tile([128, nch], F32, tag="V2")
    nc.vector.tensor_mul(V2[:], VAL[:], VAL[:])

    n = float(npts)
    st = small.tile([1, 12], F32, tag="st")
    outsb = small.tile([1, 2], F32, tag="outsb")
    with tc.tile_pool(name="cspsum", bufs=1, space="PSUM") as csp:
        cs = csp.tile([1, 3 * nch], F32, tag="cs")
        nc.tensor.matmul(cs[:, 0:nch], ones[:], D[:], start=True, stop=True)
        nc.tensor.matmul(cs[:, nch:2 * nch], ones[:], VAL[:], start=True, stop=True)
        nc.tensor.matmul(cs[:, 2 * nch:3 * nch], ones[:], V2[:], start=True, stop=True)
        nc.vector.tensor_reduce(
            st[:, 0:3], cs[:].rearrange("o (g x) -> o g x", g=3),
            axis=AX.X, op=ALU.add,
        )

        # chamfer_b
        nc.vector.tensor_scalar_mul(outsb[:, 0:1], st[:, 0:1], 1.0 / n)
        # value stats: mean, var (ddof=1), threshold
        nc.vector.tensor_scalar_mul(st[:, 3:4], st[:, 1:2], 1.0 / n)          # mean
        nc.vector.tensor_mul(st[:, 4:5], st[:, 1:2], st[:, 1:2])              # sumV^2
        nc.vector.tensor_scalar_mul(st[:, 5:6], st[:, 4:5], 1.0 / n)
        nc.vector.tensor_sub(st[:, 6:7], st[:, 2:3], st[:, 5:6])
        nc.vector.tensor_scalar_mul(st[:, 7:8], st[:, 6:7], 1.0 / (n - 1.0))  # var
        nc.scalar.sqrt(st[:, 8:9], st[:, 7:8])                                # std
        nc.vector.tensor_scalar_mul(st[:, 9:10], st[:, 8:9], ALPHA)
        nc.vector.tensor_add(st[:, 10:11], st[:, 3:4], st[:, 9:10])           # thr

        thrb = small.tile([128, 1], F32, tag="thrb")
        nc.gpsimd.partition_broadcast(thrb[:], st[:, 10:11])
        G = acc.tile([128, nch], F32, tag="G")
        nc.vector.tensor_scalar(G[:], VAL[:], thrb[:, 0:1], None, op0=ALU.is_gt)
        VM = acc.tile([128, nch], F32, tag="VM")
        nc.vector.tensor_mul(VM[:], VAL[:], G[:])
        cs2 = csp.tile([1, nch], F32, tag="cs2")
        nc.tensor.matmul(cs2[:, 0:nch], ones[:], VM[:], start=True, stop=True)
        nc.vector.tensor_reduce(st[:, 11:12], cs2[:, 0:nch], axis=AX.X, op=ALU.add)
        nc.vector.tensor_scalar_mul(outsb[:, 1:2], st[:, 11:12], 1.0 / n)

    nc.sync.dma_start(out=out[0:1, 0:2], in_=outsb[:])


def build_nc(npts=NPTS):
    nc = bacc.Bacc("TRN2", target_bir_lowering=False, debug=False)
    adv = nc.dram_tensor("adv", [npts, 3], F32, kind="ExternalInput")
    ori = nc.dram_tensor("ori", [npts, 3], F32, kind="ExternalInput")
    out = nc.dram_tensor("out", [1, 2], F32, kind="ExternalOutput")
    with tile.TileContext(nc) as tc, ExitStack() as ctx:
        build_body(tc, ctx, adv.ap(), ori.ap(), out.ap(), npts)
    nc.compile()
    return nc


_NC_CACHE = {}


def _get_nc(npts=NPTS):
    if npts not in _NC_CACHE:
        _NC_CACHE[npts] = build_nc(npts)
    return _NC_CACHE[npts]


def kernel(**inputs) -> np.ndarray:
    from concourse.bass_utils import run_bass_kernel_spmd

    adv = np.ascontiguousarray(np.asarray(inputs["adv_pc"], dtype=np.float32))
    ori = np.ascontiguousarray(np.asarray(inputs["ori_pc"], dtype=np.float32))
    B = adv.shape[0]
    assert B == N_CORES and adv.shape[1] == NPTS, (adv.shape, ori.shape)

    nc = _get_nc()
    in_maps = [{"adv": adv[b], "ori": ori[b]} for b in range(B)]
    res = run_bass_kernel_spmd(nc, in_maps, core_ids=list(range(N_CORES)))
    parts = np.stack([r["out"][0] for r in res.results])  # [B, 2]
    loss = W_CHAMFER * parts[:, 0].mean() + W_KNN * parts[:, 1].mean()
    return np.float32(loss)
